# revision 31
# baseline (speedup 1.0000x reference)
"""AdaptiveLSTMCellWithRes on 8 TRN2 NeuronCores.

Data-parallel over batch (1024 rows/core), weights replicated.
All on-chip compute happens in transposed-activation space [feat, batch].
Mixed precision:
  - i, s, c_hat, a1, r1, r2, r3 matmuls run fp8(e4m3) with DoubleRow
    perf mode (2 k-tiles per PE pass, 2x the bf16 rate). Weights are
    pre-scaled by 64 on host (0.02-std values would land subnormal in
    e4m3); the 1/64 folds into the PSUM-evicting activation's scale.
  - f, o (the error-critical gates: f multiplies c_prev, o multiplies
    tanh(c) directly) run bf16.
  - PSUM, biases and the elementwise combine stay fp32; h/c outputs
    are written bf16 (well inside the error budget, halves store DMA).
Schedule: PE pass count is the floor (~212ns per 512-wide pass), so
everything else hides behind it:
  - Sync queue: the fp8 xh stream first (cold-start critical), then
    in-loop a1/phase-B weight slabs.
  - GpSimd queue: a1's first slab, biases, then ALL r1/r2/r3 fp8 slabs
    prefetched during phase A1 (they were arriving late when loaded
    just-in-time), then the dep-gated bulk bf16 activation loads, the
    alpha DRAM roundtrip, and all output stores (keeps ScalarE free
    for PSUM evictions, which the tail chain is latency-bound on).
  - ScalarE: PSUM evictions only. Dummy Relu/Sigmoid/Tanh activations
    run in the prologue so both ACT_TABLE_LOADs (1.3us each) happen
    before the matmul stream, not in the middle of it.
  - PE warm-up: 4 dummy bf16 matmuls on zeroed scratch keep the HAM
    activity window busy from ~7.3us while the first transfers land;
    real a1 work starts ~9us (vs 14.4us with the old 16-warmup
    prologue) and the clock is at 2.4GHz by ~10.7us.
"""

import os
import sys

if "/opt/trn_rl_repo" not in sys.path:
    sys.path.insert(0, "/opt/trn_rl_repo")

import numpy as np
import ml_dtypes

P = 128
B = 8192          # global batch
NCORES = 8
BL = B // NCORES  # batch per core (1024)
D = 1024          # feature dim
K2 = 2048         # concat(x, h) contraction
JC = D // P       # 8 output-feature tiles
KC2 = K2 // P     # 16 k-chunks for gates/a1
KC1 = D // P      # 8 k-chunks for residual/a2
TC2 = KC2 // 2    # 8 fp8 double-row pair steps
NH = BL // 2      # moving free dim per matmul (512)
WS = 64.0         # fp8 weight pre-scale
QK = 4            # k-chunks per packed activation quarter-tile
NWARM = 12        # PE warm-up matmuls: bridge the HAM activity window
                  # across the xh8-stream arrival (~13us) so the clock
                  # never drops back to 1.2GHz after the j0 data wait

# r3 precision: "full8" (all 8 k-tiles fp8 DR), "half8" (k 0..3 fp8,
# k 4..7 bf16), "bf16" (all bf16)
R3_MODE = os.environ.get("R3_MODE", "full8")

E4NP = ml_dtypes.float8_e4m3
BFNP = ml_dtypes.bfloat16

_CACHE = {}


def _build(r3_mode):
    import concourse.bass as bass  # noqa: F401
    from concourse import bacc, mybir
    import concourse.tile as tile

    F32 = mybir.dt.float32
    BF16 = mybir.dt.bfloat16
    FP8 = mybir.dt.float8e4
    AF = mybir.ActivationFunctionType
    DR = mybir.MatmulPerfMode.DoubleRow

    nc = bacc.Bacc()

    # fp8 gates (c, s, i, a1): pack[g, j, p, kk, m] = e4m3(WS * Wg[j*128+m, kk*128+p])
    w8 = nc.declare_dram_parameter("w8", [4, JC, P, KC2, P], FP8, isOutput=False)
    # fp8 r1/r2 weights: partition-first pack so the whole matrix loads
    # as ONE fully-contiguous-per-partition DMA
    w8r1 = nc.declare_dram_parameter("w8r1", [P, JC, KC1, P], FP8,
                                     isOutput=False)
    w8r2 = nc.declare_dram_parameter("w8r2", [P, JC, KC1, P], FP8,
                                     isOutput=False)
    if r3_mode == "full8":
        w8r3 = nc.declare_dram_parameter("w8r3", [P, JC, KC1, P], FP8,
                                         isOutput=False)
    elif r3_mode == "half8":
        w8r3 = nc.declare_dram_parameter("w8r3", [P, JC, KC1 // 2, P], FP8,
                                         isOutput=False)
        # bf16 half, pre-scaled by WS so it shares r3's PSUM scale
        wr3h = nc.declare_dram_parameter("wr3h", [JC, P, (KC1 // 2) * P],
                                         BF16, isOutput=False)
    else:
        w8r3 = None
        wr3 = nc.declare_dram_parameter("wr3", [JC, P, D], BF16,
                                        isOutput=False)
    # bf16 gates (f, o): pack[g, j, p, k*128+m] = W[j*128+m, k*128+p]
    wf = nc.declare_dram_parameter("wf", [2, JC, P, K2], BF16, isOutput=False)
    # a2 weight: [P, KC1, 16] e4m3; col 0 holds WS * a2_w[0, k*128+p],
    # cols 1-15 are zero padding (DoubleRow LDWEIGHTS requires the pair
    # step to be a multiple of 16 bytes — s3_lw dual-fp8 restriction)
    a2p = nc.declare_dram_parameter("a2p", [P, KC1, 16], FP8, isOutput=False)
    # biases: [P, 10*JC]; col v*JC+j holds vec_v[j*128:(j+1)*128]
    # v: 0..4 = combined gate biases (i,f,o,c,s), 5=a1_b, 6=r1_b, 7=r2_b,
    # 8=r3_b, 9=a2_b (replicated)
    biasp = nc.declare_dram_parameter("biasp", [P, 10 * JC], F32, isOutput=False)
    # activations pre-swizzled into quarter tiles: [q, p, kk, n] =
    # act[(q*QK+kk)*128+p, n]; q 0..1 = x^T, 2..3 = h^T
    xh16s = nc.declare_dram_parameter("xh16s", [4, P, QK, BL], BF16,
                                      isOutput=False)
    xh8s = nc.declare_dram_parameter("xh8s", [4, P, QK, BL], FP8,
                                     isOutput=False)
    cTs = nc.declare_dram_parameter("cTs", [2, P, QK, BL], BF16, isOutput=False)
    # out[d, 0, n] = h_t^T, out[d, 1, n] = c_t^T (bf16)
    out = nc.declare_dram_parameter("out", [D, 2, BL], BF16, isOutput=True)

    alpha_dram = nc.dram_tensor("alpha_dram", [1, BL], F32)

    with tile.TileContext(nc) as tc:
        with (
            tc.tile_pool(name="consts", bufs=1) as consts,
            tc.tile_pool(name="xh8", bufs=1) as xh8_pool,
            tc.tile_pool(name="xh16", bufs=1) as xh16_pool,
            tc.tile_pool(name="cpre", bufs=1) as cp_pool,
            tc.tile_pool(name="w8p", bufs=5) as w8_pool,
            tc.tile_pool(name="wfp", bufs=4) as wf_pool,
            tc.tile_pool(name="a1s", bufs=4) as a1_pool,
            tc.tile_pool(name="r1", bufs=1) as r1_pool,
            tc.tile_pool(name="r2", bufs=1) as r2_pool,
            tc.tile_pool(name="gates", bufs=1) as g_pool,
            tc.tile_pool(name="ew", bufs=2) as ew_pool,
            tc.tile_pool(name="psum", bufs=3, space="PSUM") as psum_pool,
            tc.tile_pool(name="psum_a2", bufs=1, space="PSUM") as psum_a2_pool,
        ):
            bias_sb = consts.tile([P, 10 * JC], F32, name="bias_sb")
            a2_sb = consts.tile([P, KC1, 16], FP8, name="a2_sb")

            def bias_ap(v, j):
                return bias_sb[:, v * JC + j: v * JC + j + 1]

            # ---- prefix. PE warm-up first: dummy bf16 matmuls on
            # vector-zeroed scratch keep the HAM activity window busy
            # from ~7.3us (right after the framework preamble) while
            # the first transfers land. They write complete start/stop
            # groups into the a2 bank, which the real a2 accumulation
            # later re-zeroes (start=True).
            ps_a2 = [psum_a2_pool.tile([16, NH], F32, tag="a20", name="psa20"),
                     psum_a2_pool.tile([16, NH], F32, tag="a21", name="psa21")]
            scr_s = consts.tile([P, 2], BF16, name="scr_s")
            scr_m = consts.tile([P, NH], BF16, name="scr_m")
            nc.vector.memzero(scr_s[:])
            nc.vector.memzero(scr_m[:])
            for _ in range(NWARM):
                nc.tensor.matmul(ps_a2[0][0:1, :], scr_s[:, 0:1], scr_m[:],
                                 start=True, stop=True)
            # dummy activations: force both ACT_TABLE_LOADs (~1.3us
            # each on ScalarE) into the prologue shadow
            dum = consts.tile([P, 2], F32, name="dum")
            nc.scalar.activation(dum[:], scr_s[:], AF.Relu)
            nc.scalar.activation(dum[:], scr_s[:], AF.Sigmoid)
            nc.scalar.activation(dum[:], scr_s[:], AF.Tanh)

            # Sync queue: the fp8 xh stream, cold-start critical. First
            # quarter split in two so the first matmul's moving data
            # lands sooner on the cold DMA path.
            x8a = xh8_pool.tile([P, 2, BL], FP8, tag="x8a", name="x8a")
            nc.sync.dma_start(out=x8a[:], in_=xh8s[0][:, 0:2, :])
            x8b = xh8_pool.tile([P, 2, BL], FP8, tag="x8b", name="x8b")
            nc.sync.dma_start(out=x8b[:], in_=xh8s[0][:, 2:4, :])
            xh8q = [None]
            for q in range(1, 4):
                tl = xh8_pool.tile([P, QK, BL], FP8, tag=f"x8{q}", name=f"x8{q}")
                nc.sync.dma_start(out=tl[:], in_=xh8s[q])
                xh8q.append(tl)

            # GpSimd queue: a1's first weight slab (needed ~9us) and the
            # small consts. The full r1/r2/r3 matrices follow as ONE
            # contiguous DMA each, dep-gated behind the first a1
            # eviction so they don't steal HBM from the critical x8 /
            # a1-slab streams that pace the start of phase A1.
            a1w0 = w8_pool.tile([P, KC2, P], FP8, tag="w8", name="w8_a1_0")
            nc.gpsimd.dma_start(out=a1w0[:], in_=w8[3, 0])
            nc.gpsimd.dma_start(out=bias_sb[:], in_=biasp[:, :])
            nc.gpsimd.dma_start(out=a2_sb[:], in_=a2p[:, :])
            r1big = consts.tile([P, JC, KC1, P], FP8, name="r1big")
            r2big = consts.tile([P, JC, KC1, P], FP8, name="r2big")
            r3big = None
            if r3_mode == "full8":
                r3big = consts.tile([P, JC, KC1, P], FP8, name="r3big")
            elif r3_mode == "half8":
                r3big = consts.tile([P, JC, KC1 // 2, P], FP8, name="r3big")

            def load_rbigs(dep_ap):
                for big, src in ((r1big, w8r1), (r2big, w8r2),
                                 (r3big, w8r3 if r3_mode != "bf16" else None)):
                    if big is None:
                        continue
                    nc.vector.tensor_copy(big[0:1, 0, 0, 0:1], dep_ap)
                    nc.gpsimd.dma_start(out=big[:], in_=src[:])

            xh16q = [None] * 4
            cpq = []

            def load_bulk_xh16(dep_ap):
                # bulk bf16 loads for phase B on the gpsimd queue. The
                # scheduler orders DMAs by dependency, not program
                # order, so a tiny write sourced from a phase-A product
                # (overwritten by the DMA) holds these transfers back
                # until the critical fp8 prefix stream has landed.
                for q in (0, 1, 2, 3):
                    tl = xh16_pool.tile([P, QK, BL], BF16, tag=f"x{q}",
                                        name=f"x{q}")
                    nc.vector.tensor_copy(tl[0:1, 0, 0:1], dep_ap)
                    nc.gpsimd.dma_start(out=tl[:], in_=xh16s[q])
                    xh16q[q] = tl

            def load_bulk_cp(dep_ap):
                for q in range(2):
                    tl = cp_pool.tile([P, QK, BL], BF16, tag=f"cp{q}",
                                      name=f"cp{q}")
                    nc.vector.tensor_copy(tl[0:1, 0, 0:1], dep_ap)
                    nc.gpsimd.dma_start(out=tl[:], in_=cTs[q])
                    cpq.append(tl)

            def rhs_xh8(t, mv):
                if t == 0:
                    return x8a[:, :, mv]
                if t == 1:
                    return x8b[:, :, mv]
                q, kk = divmod(2 * t, QK)
                return xh8q[q][:, kk:kk + 2, mv]

            def rhs_xh16(k, mv):
                q, kk = divmod(k, QK)
                return xh16q[q][:, kk:kk + 1, mv]

            def load_w8(g, j):
                wt = w8_pool.tile([P, KC2, P], FP8, tag="w8", name=f"w8_{g}_{j}")
                nc.sync.dma_start(out=wt[:], in_=w8[g, j])
                return wt

            def load_wg16(g, j):
                wt = wf_pool.tile([P, K2], BF16, tag="wg", name=f"wg_{g}_{j}",
                                  bufs=3)
                nc.sync.dma_start(out=wt[:], in_=wf[g, j])
                return wt

            def ps_pair(nm):
                return [psum_pool.tile([P, NH], F32, tag="ps0", name=f"{nm}0"),
                        psum_pool.tile([P, NH], F32, tag="ps1", name=f"{nm}1")]

            # All matmul groups run bh-OUTER with the half evicted as
            # soon as its accumulation stops: the PSUM ring is only 3
            # groups deep, and a new group's start-matmul waits on the
            # 3-back group's last eviction — evicting at half-group
            # keeps that wait off the PE (LDWEIGHTS re-loads per half
            # are fully hidden under the 216ns passes).
            def mm8(ps2, wt, tc=TC2, t0=0, evict=None):
                for bh in range(2):
                    mv = slice(bh * NH, (bh + 1) * NH)
                    for t in range(tc):
                        nc.tensor.matmul(
                            ps2[bh][:], wt[:, 2 * t:2 * t + 2, :],
                            rhs_xh8(t0 + t, mv),
                            start=(t == 0), stop=(t == tc - 1),
                            perf_mode=DR)
                    if evict is not None:
                        evict(bh)

            def mm16(ps2, wt, rhs, kc, koff=0, evict=None):
                for bh in range(2):
                    mv = slice(bh * NH, (bh + 1) * NH)
                    for k in range(kc):
                        nc.tensor.matmul(
                            ps2[bh][:], wt[:, k * P:(k + 1) * P],
                            rhs(koff + k, mv),
                            start=(k == 0), stop=(k == kc - 1))
                    if evict is not None:
                        evict(bh)

            # ---- phase A1: a1 (fp8), deferred a2 matmuls. a1 evicts
            # e4m3 into DoubleRow pair tiles (slot = j parity) so the
            # a2 contraction runs DR too: 8 passes instead of 16, and
            # half the non-DR mode switches in the stream. ----
            pend = []

            def flush_a2():
                tp, pair = pend.pop(0)
                for bh in range(2):
                    nc.tensor.matmul(ps_a2[bh][:],
                                     a2_sb[:, 2 * tp:2 * tp + 2, :],
                                     pair[bh][:, :, :], start=(tp == 0),
                                     stop=(tp == JC // 2 - 1), perf_mode=DR)

            cur_pair = None
            for j in range(JC):
                wt = a1w0 if j == 0 else load_w8(3, j)
                ps2 = ps_pair("ps_a1_")
                if j % 2 == 0:
                    cur_pair = [a1_pool.tile([P, 2, NH], FP8, tag=f"a1p{bh}",
                                             name="a1p", bufs=3)
                                for bh in range(2)]

                def ev_a1(bh, j=j, cp=cur_pair, ps2=ps2):
                    nc.scalar.activation(cp[bh][:, j % 2, :], ps2[bh][:],
                                         AF.Relu, bias=bias_ap(5, j),
                                         scale=1.0 / WS)
                mm8(ps2, wt, evict=ev_a1)
                if j % 2 == 1:
                    pend.append((j // 2, cur_pair))
                # defer the a2 matmuls one pair so PE never waits on ScalarE
                if len(pend) == 2:
                    flush_a2()
                # staged bulk prefetch, ordered by when each wave is
                # needed; each is dep-gated so HBM stays clear for the
                # stream that paces the current phase
                if j == 0:
                    load_rbigs(cur_pair[0][0:1, 0, 0:1])
                if j == 4:
                    load_bulk_xh16(cur_pair[0][0:1, 0, 0:1])
                if j == 6:
                    load_bulk_cp(cur_pair[0][0:1, 0, 0:1])
            while pend:
                flush_a2()

            # alpha = sigmoid(a2 @ a1relu + a2_b): [1, BL]; broadcast via
            # DRAM roundtrip that hides under the r1/r2 phases. The 1/WS
            # a2-weight prescale folds into the eviction scale.
            for bh in range(2):
                asb = a1_pool.tile([1, NH], F32, tag="asb", name="alpha_sb")
                nc.scalar.activation(asb[:], ps_a2[bh][0:1, :], AF.Sigmoid,
                                     bias=bias_sb[0:1, 9 * JC: 9 * JC + 1],
                                     scale=1.0 / WS)
                nc.gpsimd.dma_start(
                    out=alpha_dram[0:1, bh * NH:(bh + 1) * NH], in_=asb[:])
            alpha_rep = consts.tile([P, BL], F32, name="alpha_rep")
            nc.gpsimd.dma_start(
                out=alpha_rep[:], in_=alpha_dram[0:1, :].broadcast_to([P, BL]))

            # ---- phase A2: r1 = relu(h @ r1_w.T + b) in fp8 (moving = the
            # resident fp8 h-half). r1 evicts straight to e4m3 pair-tiles
            # so r2 can also run fp8 DoubleRow. ----
            r1q = [r1_pool.tile([P, QK, BL], FP8, tag="r1a", name="r1a"),
                   r1_pool.tile([P, QK, BL], FP8, tag="r1b", name="r1b")]
            for j in range(JC):
                ps2 = ps_pair("ps_r1_")
                qq, jj = divmod(j, QK)
                for bh in range(2):
                    mv = slice(bh * NH, (bh + 1) * NH)
                    for t in range(KC1 // 2):
                        nc.tensor.matmul(
                            ps2[bh][:], r1big[:, j, 2 * t:2 * t + 2, :],
                            rhs_xh8(TC2 // 2 + t, mv),
                            start=(t == 0), stop=(t == KC1 // 2 - 1),
                            perf_mode=DR)
                    nc.scalar.activation(
                        r1q[qq][:, jj, mv],
                        ps2[bh][:], AF.Relu, bias=bias_ap(6, j),
                        scale=1.0 / WS)

            # ---- phase A3: r2 = relu(r1 @ r2_w.T + b) in fp8 DoubleRow.
            # Eviction dtype depends on r3's precision: e4m3 quarter
            # tiles where r3 runs DoubleRow, bf16 where it runs bf16.
            n8 = {"full8": JC, "half8": QK, "bf16": 0}[r3_mode]
            r2q8 = []
            if n8:
                r2q8 = [r1_pool.tile([P, QK, BL], FP8, tag="r2qa", name="r2qa")]
                if n8 > QK:
                    r2q8.append(
                        r1_pool.tile([P, QK, BL], FP8, tag="r2qb", name="r2qb"))
            r2 = []
            for j in range(JC):
                ps2 = ps_pair("ps_r2_")
                qq, jj = divmod(j, QK)
                t_ = None
                if j >= n8:
                    t_ = r2_pool.tile([P, BL], BF16, tag=f"r2_{j}",
                                      name=f"r2_{j}")
                    r2.append(t_)
                for bh in range(2):
                    mv = slice(bh * NH, (bh + 1) * NH)
                    for t in range(KC1 // 2):
                        nc.tensor.matmul(
                            ps2[bh][:], r2big[:, j, 2 * t:2 * t + 2, :],
                            r1q[t // 2][:, (2 * t) % QK:(2 * t) % QK + 2, mv],
                            start=(t == 0), stop=(t == KC1 // 2 - 1),
                            perf_mode=DR)
                    dst = (r2q8[qq][:, jj, mv] if j < n8 else t_[:, mv])
                    nc.scalar.activation(dst, ps2[bh][:], AF.Relu,
                                         bias=bias_ap(7, j), scale=1.0 / WS)

            # ---- phase B: gates + r3 + combine, per feature tile j.
            # Order c,s,i (fp8), o, f (bf16), r3: the elementwise chain runs
            # while later matmuls stream; o comes before f/r3 so only the
            # short r3-evict -> add -> tanh -> mul chain trails the last MM.
            GATE8 = {"c": (0, 3, AF.Tanh), "s": (1, 4, AF.Sigmoid),
                     "i": (2, 0, AF.Sigmoid)}

            def gate8(key, j):
                gi, v, fn = GATE8[key]
                wt = load_w8(gi, j)
                t_ = g_pool.tile([P, BL], F32, tag=f"g8{key}", name=f"g8{key}")
                ps2 = ps_pair("ps_g8")

                def ev(bh):
                    mv = slice(bh * NH, (bh + 1) * NH)
                    nc.scalar.activation(t_[:, mv], ps2[bh][:], fn,
                                         bias=bias_ap(v, j), scale=1.0 / WS)
                mm8(ps2, wt, evict=ev)
                return t_

            def gate16(gi, v, j):
                wt = load_wg16(gi, j)
                t_ = g_pool.tile([P, BL], F32, tag=f"g16{gi}", name=f"g16{gi}")
                ps2 = ps_pair("ps_g16")

                def ev(bh):
                    mv = slice(bh * NH, (bh + 1) * NH)
                    nc.scalar.activation(t_[:, mv], ps2[bh][:], AF.Sigmoid,
                                         bias=bias_ap(v, j))
                mm16(ps2, wt, rhs_xh16, KC2, evict=ev)
                return t_

            def rhs_r2q8(t, mv):
                # DR pair t over the fp8 r2 quarters
                return r2q8[t // 2][:, (2 * t) % QK:(2 * t) % QK + 2, mv]

            def rhs_r2_16(k, mv):
                # bf16 r2 tiles; in half8 mode tile list starts at k=QK
                return r2[k - (QK if r3_mode == "half8" else 0)][:, mv]

            def mm_r3(ps2, j, wt3b, evict):
                # r3 accumulation: fp8 DR pairs then bf16 k-tiles (both
                # weight halves pre-scaled by WS so PSUM shares one scale)
                n_dr = {"full8": KC1 // 2, "half8": KC1 // 4, "bf16": 0}[r3_mode]
                n_16 = KC1 - 2 * n_dr
                for bh in range(2):
                    mv = slice(bh * NH, (bh + 1) * NH)
                    for t in range(n_dr):
                        nc.tensor.matmul(
                            ps2[bh][:], r3big[:, j, 2 * t:2 * t + 2, :],
                            rhs_r2q8(t, mv), start=(t == 0),
                            stop=(t == n_dr - 1 and n_16 == 0),
                            perf_mode=DR)
                    for k in range(n_16):
                        nc.tensor.matmul(
                            ps2[bh][:], wt3b[:, k * P:(k + 1) * P],
                            rhs_r2_16(2 * n_dr + k, mv),
                            start=(k == 0 and n_dr == 0),
                            stop=(k == n_16 - 1))
                    evict(bh)

            for j in range(JC):
                last = j == JC - 1
                ch = gate8("c", j)
                st = gate8("s", j)
                it = gate8("i", j)

                t1s = []
                for bh in range(2):
                    mv = slice(bh * NH, (bh + 1) * NH)
                    t1 = ew_pool.tile([P, NH], F32, tag=f"t1{bh}", name="t1")
                    nc.vector.tensor_mul(t1[:], it[:, mv], ch[:, mv])
                    nc.vector.tensor_mul(t1[:], t1[:], st[:, mv])
                    nc.vector.tensor_mul(t1[:], t1[:], alpha_rep[:, mv])
                    t1s.append(t1)

                # last j: f before o, so f's evict -> t2 -> t1 chain
                # hides under o's and r3's matmuls and only the short
                # r3-evict -> add -> tanh -> mul chain trails the last MM
                if last:
                    ft = gate16(0, 1, j)
                    ot = gate16(1, 2, j)
                else:
                    ot = gate16(1, 2, j)
                    ft = gate16(0, 1, j)
                for bh in range(2):
                    mv = slice(bh * NH, (bh + 1) * NH)
                    qq, kk = divmod(j, QK)
                    t2 = ew_pool.tile([P, NH], F32, tag=f"t2{bh}", name="t2",
                                      bufs=1)
                    nc.vector.tensor_mul(t2[:], ft[:, mv],
                                         cpq[qq][:, kk, mv])
                    nc.vector.tensor_add(t1s[bh][:], t1s[bh][:], t2[:])

                if r3_mode == "bf16":
                    wt3b = wf_pool.tile([P, D], BF16, tag="wr", bufs=3,
                                        name=f"wr3_{j}")
                    nc.sync.dma_start(out=wt3b[:], in_=wr3[j])
                elif r3_mode == "half8":
                    wt3b = wf_pool.tile([P, (KC1 // 2) * P], BF16, tag="wr",
                                        bufs=3, name=f"wr3h_{j}")
                    nc.sync.dma_start(out=wt3b[:], in_=wr3h[j])
                else:
                    wt3b = None
                ps2 = ps_pair("ps_r3_")
                # stage[:, 0, :] = h, stage[:, 1, :] = c -> single store.
                # The finish chain runs per batch half right after that
                # half's accumulation stops (overlapping the other
                # half's matmuls); the last j additionally splits into
                # NH/2 chunks to pipeline evict -> add -> tanh -> mul.
                r3scale = 1.0 if r3_mode == "bf16" else 1.0 / WS
                nch = 1 if not last else 2
                hw_ = NH // nch

                def finish(bh):
                    for cc in range(nch):
                        pv = slice(cc * hw_, (cc + 1) * hw_)
                        mv = slice(bh * NH + cc * hw_,
                                   bh * NH + (cc + 1) * hw_)
                        stg = ew_pool.tile([P, 2, hw_], BF16,
                                           tag=f"st{bh}_{nch}{cc}", name="stg")
                        if r3_mode == "bf16":
                            # c = (r3_psum + r3_bias) + t1 in one DVE op
                            # straight from PSUM
                            nc.vector.scalar_tensor_tensor(
                                stg[:, 1, :], ps2[bh][:, pv], bias_ap(8, j),
                                t1s[bh][:, pv],
                                mybir.AluOpType.add, mybir.AluOpType.add)
                        else:
                            # fp8 r3 PSUM carries the WS weight scale:
                            # evict via ScalarE (scale+bias), add on DVE
                            r3sb = ew_pool.tile([P, hw_],
                                                F32 if r3_mode == "full8"
                                                else BF16,
                                                tag=f"r3{bh}{cc}", name="r3sb",
                                                bufs=1)
                            nc.scalar.activation(r3sb[:], ps2[bh][:, pv],
                                                 AF.Identity,
                                                 bias=bias_ap(8, j),
                                                 scale=r3scale)
                            nc.vector.tensor_add(stg[:, 1, :], t1s[bh][:, pv],
                                                 r3sb[:])
                        th = ew_pool.tile([P, hw_],
                                          F32 if r3_mode == "full8" else BF16,
                                          tag=f"th{bh}{cc}",
                                          name="th", bufs=1)
                        nc.scalar.activation(th[:], stg[:, 1, :], AF.Tanh)
                        nc.vector.tensor_mul(stg[:, 0, :], ot[:, mv], th[:])
                        # last j's stores ride Sync (idle by then): a
                        # trailing gpsimd store chain made the epilogue
                        # queue-drain ~5us long
                        eng = nc.sync if last else nc.gpsimd
                        eng.dma_start(
                            out=out[j * P:(j + 1) * P, :, mv], in_=stg[:])
                mm_r3(ps2, j, wt3b, evict=finish)

    nc.finalize()
    return nc


def _pack_w(W, kdim):
    # pack[j, p, k*128+m] = W[j*128+m, k*128+p]
    kc = kdim // P
    return np.ascontiguousarray(
        np.asarray(W, np.float32).reshape(JC, P, kc, P)
        .transpose(0, 3, 2, 1).reshape(JC, P, kc * P))


def _pack_act(aT, nq, qk=QK):
    # aT: [nq*qk*P, BL] -> [nq, P, qk, BL] with [q, p, kk, n] = aT[(q*qk+kk)*P+p, n]
    return np.ascontiguousarray(
        aT.reshape(nq, qk, P, BL).transpose(0, 2, 1, 3))


def _prepare(inputs, r3_mode):
    f = lambda name: np.asarray(inputs[name], dtype=np.float32)

    def comb(g):
        u = "U" + g[1]
        return np.concatenate([f(g + "_w"), f(u + "_w")], axis=1)

    # fp8 gates: c, s, i, a1 (order matches in-kernel GATE8/a1 indices)
    w8 = np.stack([
        _pack_w(comb("Wc") * WS, K2),
        _pack_w(comb("Ws") * WS, K2),
        _pack_w(comb("Wi") * WS, K2),
        _pack_w(f("a1_w") * WS, K2),
    ]).astype(E4NP).reshape(4, JC, P, KC2, P)
    def pack_pfirst(W):
        # [P, JC, KC1, P]: partition dim first so the whole matrix is
        # one contiguous-per-partition DMA
        return np.ascontiguousarray(
            _pack_w(W, D).reshape(JC, P, KC1, P).transpose(1, 0, 2, 3))

    w8r1 = pack_pfirst(f("r1_w") * WS).astype(E4NP)
    w8r2 = pack_pfirst(f("r2_w") * WS).astype(E4NP)
    # bf16 gates: f, o
    wf_ = np.stack([_pack_w(comb("Wf"), K2),
                    _pack_w(comb("Wo"), K2)]).astype(BFNP)
    a2p = np.zeros((P, KC1, 16), np.float32)
    a2p[:, :, 0] = (f("a2_w") * WS).reshape(KC1, P).T
    a2p = a2p.astype(E4NP)

    shared = {"w8": w8, "w8r1": w8r1, "w8r2": w8r2, "wf": wf_, "a2p": a2p}
    r3pf = pack_pfirst(f("r3_w") * WS)  # [P, JC, KC1, P], scaled by WS
    if r3_mode == "full8":
        shared["w8r3"] = r3pf.astype(E4NP)
    elif r3_mode == "half8":
        shared["w8r3"] = np.ascontiguousarray(
            r3pf[:, :, :KC1 // 2]).astype(E4NP)
        shared["wr3h"] = np.ascontiguousarray(
            r3pf.transpose(1, 0, 2, 3)[:, :, KC1 // 2:]
            .reshape(JC, P, (KC1 // 2) * P)).astype(BFNP)
    else:
        shared["wr3"] = _pack_w(f("r3_w"), D).astype(BFNP)

    bias_vecs = []
    for g in ("Wi", "Wf", "Wo", "Wc", "Ws"):
        u = "U" + g[1]
        bias_vecs.append(f(g + "_b") + f(u + "_b"))
    bias_vecs += [f("a1_b"), f("r1_b"), f("r2_b"), f("r3_b"),
                  np.full(D, f("a2_b")[0], np.float32)]
    # biasp[p, v*JC + j] = vec_v[j*128 + p]
    biasp = np.ascontiguousarray(
        np.stack(bias_vecs).reshape(10, JC, P).transpose(2, 0, 1)
        .reshape(P, 10 * JC))
    shared["biasp"] = biasp

    x, h, c = f("x"), f("h_prev"), f("c_prev")
    in_maps = []
    for core in range(NCORES):
        sl = slice(core * BL, (core + 1) * BL)
        xhT = np.ascontiguousarray(
            np.concatenate([x[sl].T, h[sl].T], axis=0))  # [K2, BL]
        in_maps.append({**shared,
                        "xh16s": _pack_act(xhT.astype(BFNP), 4),
                        "xh8s": _pack_act(xhT.astype(E4NP), 4),
                        "cTs": _pack_act(
                            np.ascontiguousarray(c[sl].T).astype(BFNP), 2)})
    return in_maps


def _run(inputs, trace=False):
    from concourse.bass_utils import run_bass_kernel_spmd

    if R3_MODE not in _CACHE:
        _CACHE[R3_MODE] = _build(R3_MODE)
    nc = _CACHE[R3_MODE]
    in_maps = _prepare(inputs, R3_MODE)
    res = run_bass_kernel_spmd(nc, in_maps, core_ids=list(range(NCORES)),
                               trace=trace)
    h = np.empty((B, D), np.float32)
    c = np.empty((B, D), np.float32)
    for core in range(NCORES):
        o = res.results[core]["out"]  # [D, 2, BL] bf16
        sl = slice(core * BL, (core + 1) * BL)
        h[sl] = o[:, 0].T.astype(np.float32)
        c[sl] = o[:, 1].T.astype(np.float32)
    return (h, c), res


def kernel(**inputs):
    (h, c), _ = _run(inputs, trace=False)
    return (h, c)


# revision 33
# speedup vs baseline: 1.0195x; 1.0195x over previous
"""AdaptiveLSTMCellWithRes on 8 TRN2 NeuronCores.

Data-parallel over batch (1024 rows/core), weights replicated.
All on-chip compute happens in transposed-activation space [feat, batch].
Mixed precision:
  - i, s, c_hat, a1, r1, r2, r3 matmuls run fp8(e4m3) with DoubleRow
    perf mode (2 k-tiles per PE pass, 2x the bf16 rate). Weights are
    pre-scaled by 64 on host (0.02-std values would land subnormal in
    e4m3); the 1/64 folds into the PSUM-evicting activation's scale.
  - f, o (the error-critical gates: f multiplies c_prev, o multiplies
    tanh(c) directly) run bf16.
  - PSUM, biases and the elementwise combine stay fp32; h/c outputs
    are written bf16 (well inside the error budget, halves store DMA).
Schedule: PE pass count is the floor (~212ns per 512-wide pass), so
everything else hides behind it:
  - Sync queue: the fp8 xh stream first (cold-start critical), then
    in-loop a1/phase-B weight slabs.
  - GpSimd queue: a1's first slab, biases, then ALL r1/r2/r3 fp8 slabs
    prefetched during phase A1 (they were arriving late when loaded
    just-in-time), then the dep-gated bulk bf16 activation loads, the
    alpha DRAM roundtrip, and all output stores (keeps ScalarE free
    for PSUM evictions, which the tail chain is latency-bound on).
  - ScalarE: PSUM evictions only. Dummy Relu/Sigmoid/Tanh activations
    run in the prologue so both ACT_TABLE_LOADs (1.3us each) happen
    before the matmul stream, not in the middle of it.
  - PE warm-up: 4 dummy bf16 matmuls on zeroed scratch keep the HAM
    activity window busy from ~7.3us while the first transfers land;
    real a1 work starts ~9us (vs 14.4us with the old 16-warmup
    prologue) and the clock is at 2.4GHz by ~10.7us.
"""

import os
import sys

if "/opt/trn_rl_repo" not in sys.path:
    sys.path.insert(0, "/opt/trn_rl_repo")

import numpy as np
import ml_dtypes

P = 128
B = 8192          # global batch
NCORES = 8
BL = B // NCORES  # batch per core (1024)
D = 1024          # feature dim
K2 = 2048         # concat(x, h) contraction
JC = D // P       # 8 output-feature tiles
KC2 = K2 // P     # 16 k-chunks for gates/a1
KC1 = D // P      # 8 k-chunks for residual/a2
TC2 = KC2 // 2    # 8 fp8 double-row pair steps
NH = BL // 2      # moving free dim per matmul (512)
WS = 64.0         # fp8 weight pre-scale
QK = 4            # k-chunks per packed activation quarter-tile
NWARM = 8         # PE warm-up matmuls: keep the HAM activity window
                  # busy until the first xh8 quarters land (~10.5us);
                  # j0's passes then carry the activity to full arrival

# r3 precision: "full8" (all 8 k-tiles fp8 DR), "half8" (k 0..3 fp8,
# k 4..7 bf16), "bf16" (all bf16)
R3_MODE = os.environ.get("R3_MODE", "full8")

E4NP = ml_dtypes.float8_e4m3
BFNP = ml_dtypes.bfloat16

_CACHE = {}


def _build(r3_mode):
    import concourse.bass as bass  # noqa: F401
    from concourse import bacc, mybir
    import concourse.tile as tile

    F32 = mybir.dt.float32
    BF16 = mybir.dt.bfloat16
    FP8 = mybir.dt.float8e4
    AF = mybir.ActivationFunctionType
    DR = mybir.MatmulPerfMode.DoubleRow

    nc = bacc.Bacc()

    # fp8 gates (c, s, i, a1): pack[g, j, p, kk, m] = e4m3(WS * Wg[j*128+m, kk*128+p])
    w8 = nc.declare_dram_parameter("w8", [4, JC, P, KC2, P], FP8, isOutput=False)
    # fp8 r1/r2 weights: partition-first pack so the whole matrix loads
    # as ONE fully-contiguous-per-partition DMA
    w8r1 = nc.declare_dram_parameter("w8r1", [P, JC, KC1, P], FP8,
                                     isOutput=False)
    w8r2 = nc.declare_dram_parameter("w8r2", [P, JC, KC1, P], FP8,
                                     isOutput=False)
    if r3_mode == "full8":
        w8r3 = nc.declare_dram_parameter("w8r3", [P, JC, KC1, P], FP8,
                                         isOutput=False)
    elif r3_mode == "half8":
        w8r3 = nc.declare_dram_parameter("w8r3", [P, JC, KC1 // 2, P], FP8,
                                         isOutput=False)
        # bf16 half, pre-scaled by WS so it shares r3's PSUM scale
        wr3h = nc.declare_dram_parameter("wr3h", [JC, P, (KC1 // 2) * P],
                                         BF16, isOutput=False)
    else:
        w8r3 = None
        wr3 = nc.declare_dram_parameter("wr3", [JC, P, D], BF16,
                                        isOutput=False)
    # bf16 gates (f, o): pack[g, j, p, k*128+m] = W[j*128+m, k*128+p]
    wf = nc.declare_dram_parameter("wf", [2, JC, P, K2], BF16, isOutput=False)
    # a2 weight: [P, KC1, 16] e4m3; col 0 holds WS * a2_w[0, k*128+p],
    # cols 1-15 are zero padding (DoubleRow LDWEIGHTS requires the pair
    # step to be a multiple of 16 bytes — s3_lw dual-fp8 restriction)
    a2p = nc.declare_dram_parameter("a2p", [P, KC1, 16], FP8, isOutput=False)
    # biases: [P, 10*JC]; col v*JC+j holds vec_v[j*128:(j+1)*128]
    # v: 0..4 = combined gate biases (i,f,o,c,s), 5=a1_b, 6=r1_b, 7=r2_b,
    # 8=r3_b, 9=a2_b (replicated)
    biasp = nc.declare_dram_parameter("biasp", [P, 10 * JC], F32, isOutput=False)
    # activations pre-swizzled into quarter tiles: [q, p, kk, n] =
    # act[(q*QK+kk)*128+p, n]; q 0..1 = x^T, 2..3 = h^T
    xh16s = nc.declare_dram_parameter("xh16s", [4, P, QK, BL], BF16,
                                      isOutput=False)
    xh8s = nc.declare_dram_parameter("xh8s", [4, P, QK, BL], FP8,
                                     isOutput=False)
    cTs = nc.declare_dram_parameter("cTs", [2, P, QK, BL], BF16, isOutput=False)
    # out[d, 0, n] = h_t^T, out[d, 1, n] = c_t^T (bf16)
    out = nc.declare_dram_parameter("out", [D, 2, BL], BF16, isOutput=True)

    alpha_dram = nc.dram_tensor("alpha_dram", [1, BL], F32)

    with tile.TileContext(nc) as tc:
        with (
            tc.tile_pool(name="consts", bufs=1) as consts,
            tc.tile_pool(name="xh8", bufs=1) as xh8_pool,
            tc.tile_pool(name="xh16", bufs=1) as xh16_pool,
            tc.tile_pool(name="cpre", bufs=1) as cp_pool,
            tc.tile_pool(name="w8p", bufs=5) as w8_pool,
            tc.tile_pool(name="wfp", bufs=4) as wf_pool,
            tc.tile_pool(name="a1s", bufs=4) as a1_pool,
            tc.tile_pool(name="r1", bufs=1) as r1_pool,
            tc.tile_pool(name="r2", bufs=1) as r2_pool,
            tc.tile_pool(name="gates", bufs=1) as g_pool,
            tc.tile_pool(name="ew", bufs=2) as ew_pool,
            tc.tile_pool(name="psum", bufs=3, space="PSUM") as psum_pool,
            tc.tile_pool(name="psum_a2", bufs=1, space="PSUM") as psum_a2_pool,
        ):
            bias_sb = consts.tile([P, 10 * JC], F32, name="bias_sb")
            a2_sb = consts.tile([P, KC1, 16], FP8, name="a2_sb")

            def bias_ap(v, j):
                return bias_sb[:, v * JC + j: v * JC + j + 1]

            # ---- prefix. PE warm-up first: dummy bf16 matmuls on
            # vector-zeroed scratch keep the HAM activity window busy
            # from ~7.3us (right after the framework preamble) while
            # the first transfers land. They write complete start/stop
            # groups into the a2 bank, which the real a2 accumulation
            # later re-zeroes (start=True).
            ps_a2 = [psum_a2_pool.tile([16, NH], F32, tag="a20", name="psa20"),
                     psum_a2_pool.tile([16, NH], F32, tag="a21", name="psa21")]
            scr_s = consts.tile([P, 2], BF16, name="scr_s")
            scr_m = consts.tile([P, NH], BF16, name="scr_m")
            nc.vector.memzero(scr_s[:])
            nc.vector.memzero(scr_m[:])
            for _ in range(NWARM):
                nc.tensor.matmul(ps_a2[0][0:1, :], scr_s[:, 0:1], scr_m[:],
                                 start=True, stop=True)
            # dummy activations: force both ACT_TABLE_LOADs (~1.3us
            # each on ScalarE) into the prologue shadow
            dum = consts.tile([P, 2], F32, name="dum")
            nc.scalar.activation(dum[:], scr_s[:], AF.Relu)
            nc.scalar.activation(dum[:], scr_s[:], AF.Sigmoid)
            nc.scalar.activation(dum[:], scr_s[:], AF.Tanh)

            # Sync queue: the fp8 xh stream, cold-start critical. First
            # quarter split in two so the first matmul's moving data
            # lands sooner on the cold DMA path.
            x8a = xh8_pool.tile([P, 2, BL], FP8, tag="x8a", name="x8a")
            nc.sync.dma_start(out=x8a[:], in_=xh8s[0][:, 0:2, :])
            x8b = xh8_pool.tile([P, 2, BL], FP8, tag="x8b", name="x8b")
            nc.sync.dma_start(out=x8b[:], in_=xh8s[0][:, 2:4, :])
            xh8q = [None]
            for q in range(1, 4):
                tl = xh8_pool.tile([P, QK, BL], FP8, tag=f"x8{q}", name=f"x8{q}")
                nc.sync.dma_start(out=tl[:], in_=xh8s[q])
                xh8q.append(tl)

            # GpSimd queue: a1's first weight slab (needed ~9us) and the
            # small consts. The full r1/r2/r3 matrices follow as ONE
            # contiguous DMA each, dep-gated behind the first a1
            # eviction so they don't steal HBM from the critical x8 /
            # a1-slab streams that pace the start of phase A1.
            a1w0 = w8_pool.tile([P, KC2, P], FP8, tag="w8", name="w8_a1_0")
            nc.gpsimd.dma_start(out=a1w0[:], in_=w8[3, 0])
            nc.gpsimd.dma_start(out=bias_sb[:], in_=biasp[:, :])
            nc.gpsimd.dma_start(out=a2_sb[:], in_=a2p[:, :])
            r1big = consts.tile([P, JC, KC1, P], FP8, name="r1big")
            r2big = consts.tile([P, JC, KC1, P], FP8, name="r2big")
            r3big = None
            if r3_mode == "full8":
                r3big = consts.tile([P, JC, KC1, P], FP8, name="r3big")
            elif r3_mode == "half8":
                r3big = consts.tile([P, JC, KC1 // 2, P], FP8, name="r3big")

            def load_rbigs(dep_ap):
                for big, src in ((r1big, w8r1), (r2big, w8r2),
                                 (r3big, w8r3 if r3_mode != "bf16" else None)):
                    if big is None:
                        continue
                    nc.vector.tensor_copy(big[0:1, 0, 0, 0:1], dep_ap)
                    nc.gpsimd.dma_start(out=big[:], in_=src[:])

            xh16q = [None] * 4
            cpq = []

            def load_bulk_xh16(dep_ap):
                # bulk bf16 loads for phase B on the gpsimd queue. The
                # scheduler orders DMAs by dependency, not program
                # order, so a tiny write sourced from a phase-A product
                # (overwritten by the DMA) holds these transfers back
                # until the critical fp8 prefix stream has landed.
                for q in (0, 1, 2, 3):
                    tl = xh16_pool.tile([P, QK, BL], BF16, tag=f"x{q}",
                                        name=f"x{q}")
                    nc.vector.tensor_copy(tl[0:1, 0, 0:1], dep_ap)
                    nc.gpsimd.dma_start(out=tl[:], in_=xh16s[q])
                    xh16q[q] = tl

            def load_bulk_cp(dep_ap):
                for q in range(2):
                    tl = cp_pool.tile([P, QK, BL], BF16, tag=f"cp{q}",
                                      name=f"cp{q}")
                    nc.vector.tensor_copy(tl[0:1, 0, 0:1], dep_ap)
                    nc.gpsimd.dma_start(out=tl[:], in_=cTs[q])
                    cpq.append(tl)

            def rhs_xh8(t, mv):
                if t == 0:
                    return x8a[:, :, mv]
                if t == 1:
                    return x8b[:, :, mv]
                q, kk = divmod(2 * t, QK)
                return xh8q[q][:, kk:kk + 2, mv]

            def rhs_xh16(k, mv):
                q, kk = divmod(k, QK)
                return xh16q[q][:, kk:kk + 1, mv]

            def load_w8(g, j):
                wt = w8_pool.tile([P, KC2, P], FP8, tag="w8", name=f"w8_{g}_{j}")
                nc.sync.dma_start(out=wt[:], in_=w8[g, j])
                return wt

            def load_wg16(g, j):
                wt = wf_pool.tile([P, K2], BF16, tag="wg", name=f"wg_{g}_{j}",
                                  bufs=3)
                nc.sync.dma_start(out=wt[:], in_=wf[g, j])
                return wt

            def ps_pair(nm):
                return [psum_pool.tile([P, NH], F32, tag="ps0", name=f"{nm}0"),
                        psum_pool.tile([P, NH], F32, tag="ps1", name=f"{nm}1")]

            # All matmul groups run bh-OUTER with the half evicted as
            # soon as its accumulation stops: the PSUM ring is only 3
            # groups deep, and a new group's start-matmul waits on the
            # 3-back group's last eviction — evicting at half-group
            # keeps that wait off the PE (LDWEIGHTS re-loads per half
            # are fully hidden under the 216ns passes).
            def mm8(ps2, wt, tc=TC2, t0=0, evict=None):
                for bh in range(2):
                    mv = slice(bh * NH, (bh + 1) * NH)
                    for t in range(tc):
                        nc.tensor.matmul(
                            ps2[bh][:], wt[:, 2 * t:2 * t + 2, :],
                            rhs_xh8(t0 + t, mv),
                            start=(t == 0), stop=(t == tc - 1),
                            perf_mode=DR)
                    if evict is not None:
                        evict(bh)

            def mm16(ps2, wt, rhs, kc, koff=0, evict=None):
                for bh in range(2):
                    mv = slice(bh * NH, (bh + 1) * NH)
                    for k in range(kc):
                        nc.tensor.matmul(
                            ps2[bh][:], wt[:, k * P:(k + 1) * P],
                            rhs(koff + k, mv),
                            start=(k == 0), stop=(k == kc - 1))
                    if evict is not None:
                        evict(bh)

            # ---- phase A1: a1 (fp8), deferred a2 matmuls. a1 evicts
            # e4m3 into DoubleRow pair tiles (slot = j parity) so the
            # a2 contraction runs DR too: 8 passes instead of 16, and
            # half the non-DR mode switches in the stream. ----
            pend = []

            def flush_a2():
                tp, pair = pend.pop(0)
                for bh in range(2):
                    nc.tensor.matmul(ps_a2[bh][:],
                                     a2_sb[:, 2 * tp:2 * tp + 2, :],
                                     pair[bh][:, :, :], start=(tp == 0),
                                     stop=(tp == JC // 2 - 1), perf_mode=DR)

            cur_pair = None
            for j in range(JC):
                wt = a1w0 if j == 0 else load_w8(3, j)
                ps2 = ps_pair("ps_a1_")
                if j % 2 == 0:
                    cur_pair = [a1_pool.tile([P, 2, NH], FP8, tag=f"a1p{bh}",
                                             name="a1p", bufs=3)
                                for bh in range(2)]

                def ev_a1(bh, j=j, cp=cur_pair, ps2=ps2):
                    nc.scalar.activation(cp[bh][:, j % 2, :], ps2[bh][:],
                                         AF.Relu, bias=bias_ap(5, j),
                                         scale=1.0 / WS)
                if j == 0:
                    # j0 is paced by the arriving xh8 stream: t-outer so
                    # only the last-quarter passes remain when q3 lands
                    for t in range(TC2):
                        for bh in range(2):
                            mv = slice(bh * NH, (bh + 1) * NH)
                            nc.tensor.matmul(
                                ps2[bh][:], wt[:, 2 * t:2 * t + 2, :],
                                rhs_xh8(t, mv), start=(t == 0),
                                stop=(t == TC2 - 1), perf_mode=DR)
                    ev_a1(0)
                    ev_a1(1)
                else:
                    mm8(ps2, wt, evict=ev_a1)
                if j % 2 == 1:
                    pend.append((j // 2, cur_pair))
                # defer the a2 matmuls one pair so PE never waits on ScalarE
                if len(pend) == 2:
                    flush_a2()
                # staged bulk prefetch, ordered by when each wave is
                # needed; each is dep-gated so HBM stays clear for the
                # stream that paces the current phase (r-bigs must not
                # steal HBM from the a1 slab stream that paces j1-j3)
                if j == 2:
                    load_rbigs(cur_pair[0][0:1, 0, 0:1])
                if j == 4:
                    load_bulk_xh16(cur_pair[0][0:1, 0, 0:1])
                if j == 6:
                    load_bulk_cp(cur_pair[0][0:1, 0, 0:1])
            while pend:
                flush_a2()

            # alpha = sigmoid(a2 @ a1relu + a2_b): [1, BL]; broadcast via
            # DRAM roundtrip that hides under the r1/r2 phases. The 1/WS
            # a2-weight prescale folds into the eviction scale.
            for bh in range(2):
                asb = a1_pool.tile([1, NH], F32, tag="asb", name="alpha_sb")
                nc.scalar.activation(asb[:], ps_a2[bh][0:1, :], AF.Sigmoid,
                                     bias=bias_sb[0:1, 9 * JC: 9 * JC + 1],
                                     scale=1.0 / WS)
                nc.gpsimd.dma_start(
                    out=alpha_dram[0:1, bh * NH:(bh + 1) * NH], in_=asb[:])
            alpha_rep = consts.tile([P, BL], F32, name="alpha_rep")
            nc.gpsimd.dma_start(
                out=alpha_rep[:], in_=alpha_dram[0:1, :].broadcast_to([P, BL]))

            # ---- phase A2: r1 = relu(h @ r1_w.T + b) in fp8 (moving = the
            # resident fp8 h-half). r1 evicts straight to e4m3 pair-tiles
            # so r2 can also run fp8 DoubleRow. ----
            r1q = [r1_pool.tile([P, QK, BL], FP8, tag="r1a", name="r1a"),
                   r1_pool.tile([P, QK, BL], FP8, tag="r1b", name="r1b")]
            for j in range(JC):
                ps2 = ps_pair("ps_r1_")
                qq, jj = divmod(j, QK)
                for bh in range(2):
                    mv = slice(bh * NH, (bh + 1) * NH)
                    for t in range(KC1 // 2):
                        nc.tensor.matmul(
                            ps2[bh][:], r1big[:, j, 2 * t:2 * t + 2, :],
                            rhs_xh8(TC2 // 2 + t, mv),
                            start=(t == 0), stop=(t == KC1 // 2 - 1),
                            perf_mode=DR)
                    nc.scalar.activation(
                        r1q[qq][:, jj, mv],
                        ps2[bh][:], AF.Relu, bias=bias_ap(6, j),
                        scale=1.0 / WS)

            # ---- phase A3: r2 = relu(r1 @ r2_w.T + b) in fp8 DoubleRow.
            # Eviction dtype depends on r3's precision: e4m3 quarter
            # tiles where r3 runs DoubleRow, bf16 where it runs bf16.
            n8 = {"full8": JC, "half8": QK, "bf16": 0}[r3_mode]
            r2q8 = []
            if n8:
                r2q8 = [r1_pool.tile([P, QK, BL], FP8, tag="r2qa", name="r2qa")]
                if n8 > QK:
                    r2q8.append(
                        r1_pool.tile([P, QK, BL], FP8, tag="r2qb", name="r2qb"))
            r2 = []
            for j in range(JC):
                ps2 = ps_pair("ps_r2_")
                qq, jj = divmod(j, QK)
                t_ = None
                if j >= n8:
                    t_ = r2_pool.tile([P, BL], BF16, tag=f"r2_{j}",
                                      name=f"r2_{j}")
                    r2.append(t_)
                for bh in range(2):
                    mv = slice(bh * NH, (bh + 1) * NH)
                    for t in range(KC1 // 2):
                        nc.tensor.matmul(
                            ps2[bh][:], r2big[:, j, 2 * t:2 * t + 2, :],
                            r1q[t // 2][:, (2 * t) % QK:(2 * t) % QK + 2, mv],
                            start=(t == 0), stop=(t == KC1 // 2 - 1),
                            perf_mode=DR)
                    dst = (r2q8[qq][:, jj, mv] if j < n8 else t_[:, mv])
                    nc.scalar.activation(dst, ps2[bh][:], AF.Relu,
                                         bias=bias_ap(7, j), scale=1.0 / WS)

            # ---- phase B: gates + r3 + combine, per feature tile j.
            # Order c,s,i (fp8), o, f (bf16), r3: the elementwise chain runs
            # while later matmuls stream; o comes before f/r3 so only the
            # short r3-evict -> add -> tanh -> mul chain trails the last MM.
            GATE8 = {"c": (0, 3, AF.Tanh), "s": (1, 4, AF.Sigmoid),
                     "i": (2, 0, AF.Sigmoid)}

            def gate8(key, j):
                gi, v, fn = GATE8[key]
                wt = load_w8(gi, j)
                t_ = g_pool.tile([P, BL], F32, tag=f"g8{key}", name=f"g8{key}")
                ps2 = ps_pair("ps_g8")

                def ev(bh):
                    mv = slice(bh * NH, (bh + 1) * NH)
                    nc.scalar.activation(t_[:, mv], ps2[bh][:], fn,
                                         bias=bias_ap(v, j), scale=1.0 / WS)
                mm8(ps2, wt, evict=ev)
                return t_

            def gate16(gi, v, j):
                wt = load_wg16(gi, j)
                t_ = g_pool.tile([P, BL], F32, tag=f"g16{gi}", name=f"g16{gi}")
                ps2 = ps_pair("ps_g16")

                def ev(bh):
                    mv = slice(bh * NH, (bh + 1) * NH)
                    nc.scalar.activation(t_[:, mv], ps2[bh][:], AF.Sigmoid,
                                         bias=bias_ap(v, j))
                mm16(ps2, wt, rhs_xh16, KC2, evict=ev)
                return t_

            def rhs_r2q8(t, mv):
                # DR pair t over the fp8 r2 quarters
                return r2q8[t // 2][:, (2 * t) % QK:(2 * t) % QK + 2, mv]

            def rhs_r2_16(k, mv):
                # bf16 r2 tiles; in half8 mode tile list starts at k=QK
                return r2[k - (QK if r3_mode == "half8" else 0)][:, mv]

            def mm_r3(ps2, j, wt3b, evict):
                # r3 accumulation: fp8 DR pairs then bf16 k-tiles (both
                # weight halves pre-scaled by WS so PSUM shares one scale)
                n_dr = {"full8": KC1 // 2, "half8": KC1 // 4, "bf16": 0}[r3_mode]
                n_16 = KC1 - 2 * n_dr
                for bh in range(2):
                    mv = slice(bh * NH, (bh + 1) * NH)
                    for t in range(n_dr):
                        nc.tensor.matmul(
                            ps2[bh][:], r3big[:, j, 2 * t:2 * t + 2, :],
                            rhs_r2q8(t, mv), start=(t == 0),
                            stop=(t == n_dr - 1 and n_16 == 0),
                            perf_mode=DR)
                    for k in range(n_16):
                        nc.tensor.matmul(
                            ps2[bh][:], wt3b[:, k * P:(k + 1) * P],
                            rhs_r2_16(2 * n_dr + k, mv),
                            start=(k == 0 and n_dr == 0),
                            stop=(k == n_16 - 1))
                    evict(bh)

            for j in range(JC):
                last = j == JC - 1
                ch = gate8("c", j)
                st = gate8("s", j)
                it = gate8("i", j)

                t1s = []
                for bh in range(2):
                    mv = slice(bh * NH, (bh + 1) * NH)
                    t1 = ew_pool.tile([P, NH], F32, tag=f"t1{bh}", name="t1")
                    nc.vector.tensor_mul(t1[:], it[:, mv], ch[:, mv])
                    nc.vector.tensor_mul(t1[:], t1[:], st[:, mv])
                    nc.vector.tensor_mul(t1[:], t1[:], alpha_rep[:, mv])
                    t1s.append(t1)

                # last j: f before o, so f's evict -> t2 -> t1 chain
                # hides under o's and r3's matmuls and only the short
                # r3-evict -> add -> tanh -> mul chain trails the last MM
                if last:
                    ft = gate16(0, 1, j)
                    ot = gate16(1, 2, j)
                else:
                    ot = gate16(1, 2, j)
                    ft = gate16(0, 1, j)
                for bh in range(2):
                    mv = slice(bh * NH, (bh + 1) * NH)
                    qq, kk = divmod(j, QK)
                    t2 = ew_pool.tile([P, NH], F32, tag=f"t2{bh}", name="t2",
                                      bufs=1)
                    nc.vector.tensor_mul(t2[:], ft[:, mv],
                                         cpq[qq][:, kk, mv])
                    nc.vector.tensor_add(t1s[bh][:], t1s[bh][:], t2[:])

                if r3_mode == "bf16":
                    wt3b = wf_pool.tile([P, D], BF16, tag="wr", bufs=3,
                                        name=f"wr3_{j}")
                    nc.sync.dma_start(out=wt3b[:], in_=wr3[j])
                elif r3_mode == "half8":
                    wt3b = wf_pool.tile([P, (KC1 // 2) * P], BF16, tag="wr",
                                        bufs=3, name=f"wr3h_{j}")
                    nc.sync.dma_start(out=wt3b[:], in_=wr3h[j])
                else:
                    wt3b = None
                ps2 = ps_pair("ps_r3_")
                # stage[:, 0, :] = h, stage[:, 1, :] = c -> single store.
                # The finish chain runs per batch half right after that
                # half's accumulation stops (overlapping the other
                # half's matmuls); the last j additionally splits into
                # NH/2 chunks to pipeline evict -> add -> tanh -> mul.
                r3scale = 1.0 if r3_mode == "bf16" else 1.0 / WS
                nch = 1 if not last else 4
                hw_ = NH // nch

                def finish(bh):
                    for cc in range(nch):
                        pv = slice(cc * hw_, (cc + 1) * hw_)
                        mv = slice(bh * NH + cc * hw_,
                                   bh * NH + (cc + 1) * hw_)
                        stg = ew_pool.tile([P, 2, hw_], BF16,
                                           tag=f"st{bh}_{nch}{cc}", name="stg")
                        if r3_mode == "bf16":
                            # c = (r3_psum + r3_bias) + t1 in one DVE op
                            # straight from PSUM
                            nc.vector.scalar_tensor_tensor(
                                stg[:, 1, :], ps2[bh][:, pv], bias_ap(8, j),
                                t1s[bh][:, pv],
                                mybir.AluOpType.add, mybir.AluOpType.add)
                        else:
                            # fp8 r3 PSUM carries the WS weight scale:
                            # evict via ScalarE (scale+bias), add on DVE
                            r3sb = ew_pool.tile([P, hw_],
                                                F32 if r3_mode == "full8"
                                                else BF16,
                                                tag=f"r3{bh}{cc}", name="r3sb",
                                                bufs=1)
                            nc.scalar.activation(r3sb[:], ps2[bh][:, pv],
                                                 AF.Identity,
                                                 bias=bias_ap(8, j),
                                                 scale=r3scale)
                            nc.vector.tensor_add(stg[:, 1, :], t1s[bh][:, pv],
                                                 r3sb[:])
                        th = ew_pool.tile([P, hw_],
                                          F32 if r3_mode == "full8" else BF16,
                                          tag=f"th{bh}{cc}",
                                          name="th", bufs=1)
                        nc.scalar.activation(th[:], stg[:, 1, :], AF.Tanh)
                        nc.vector.tensor_mul(stg[:, 0, :], ot[:, mv], th[:])
                        # last j's stores ride Sync (idle by then): a
                        # trailing gpsimd store chain made the epilogue
                        # queue-drain ~5us long
                        eng = nc.sync if last else nc.gpsimd
                        eng.dma_start(
                            out=out[j * P:(j + 1) * P, :, mv], in_=stg[:])
                mm_r3(ps2, j, wt3b, evict=finish)

    nc.finalize()
    return nc


def _pack_w(W, kdim):
    # pack[j, p, k*128+m] = W[j*128+m, k*128+p]
    kc = kdim // P
    return np.ascontiguousarray(
        np.asarray(W, np.float32).reshape(JC, P, kc, P)
        .transpose(0, 3, 2, 1).reshape(JC, P, kc * P))


def _pack_act(aT, nq, qk=QK):
    # aT: [nq*qk*P, BL] -> [nq, P, qk, BL] with [q, p, kk, n] = aT[(q*qk+kk)*P+p, n]
    return np.ascontiguousarray(
        aT.reshape(nq, qk, P, BL).transpose(0, 2, 1, 3))


def _prepare(inputs, r3_mode):
    f = lambda name: np.asarray(inputs[name], dtype=np.float32)

    def comb(g):
        u = "U" + g[1]
        return np.concatenate([f(g + "_w"), f(u + "_w")], axis=1)

    # fp8 gates: c, s, i, a1 (order matches in-kernel GATE8/a1 indices)
    w8 = np.stack([
        _pack_w(comb("Wc") * WS, K2),
        _pack_w(comb("Ws") * WS, K2),
        _pack_w(comb("Wi") * WS, K2),
        _pack_w(f("a1_w") * WS, K2),
    ]).astype(E4NP).reshape(4, JC, P, KC2, P)
    def pack_pfirst(W):
        # [P, JC, KC1, P]: partition dim first so the whole matrix is
        # one contiguous-per-partition DMA
        return np.ascontiguousarray(
            _pack_w(W, D).reshape(JC, P, KC1, P).transpose(1, 0, 2, 3))

    w8r1 = pack_pfirst(f("r1_w") * WS).astype(E4NP)
    w8r2 = pack_pfirst(f("r2_w") * WS).astype(E4NP)
    # bf16 gates: f, o
    wf_ = np.stack([_pack_w(comb("Wf"), K2),
                    _pack_w(comb("Wo"), K2)]).astype(BFNP)
    a2p = np.zeros((P, KC1, 16), np.float32)
    a2p[:, :, 0] = (f("a2_w") * WS).reshape(KC1, P).T
    a2p = a2p.astype(E4NP)

    shared = {"w8": w8, "w8r1": w8r1, "w8r2": w8r2, "wf": wf_, "a2p": a2p}
    r3pf = pack_pfirst(f("r3_w") * WS)  # [P, JC, KC1, P], scaled by WS
    if r3_mode == "full8":
        shared["w8r3"] = r3pf.astype(E4NP)
    elif r3_mode == "half8":
        shared["w8r3"] = np.ascontiguousarray(
            r3pf[:, :, :KC1 // 2]).astype(E4NP)
        shared["wr3h"] = np.ascontiguousarray(
            r3pf.transpose(1, 0, 2, 3)[:, :, KC1 // 2:]
            .reshape(JC, P, (KC1 // 2) * P)).astype(BFNP)
    else:
        shared["wr3"] = _pack_w(f("r3_w"), D).astype(BFNP)

    bias_vecs = []
    for g in ("Wi", "Wf", "Wo", "Wc", "Ws"):
        u = "U" + g[1]
        bias_vecs.append(f(g + "_b") + f(u + "_b"))
    bias_vecs += [f("a1_b"), f("r1_b"), f("r2_b"), f("r3_b"),
                  np.full(D, f("a2_b")[0], np.float32)]
    # biasp[p, v*JC + j] = vec_v[j*128 + p]
    biasp = np.ascontiguousarray(
        np.stack(bias_vecs).reshape(10, JC, P).transpose(2, 0, 1)
        .reshape(P, 10 * JC))
    shared["biasp"] = biasp

    x, h, c = f("x"), f("h_prev"), f("c_prev")
    in_maps = []
    for core in range(NCORES):
        sl = slice(core * BL, (core + 1) * BL)
        xhT = np.ascontiguousarray(
            np.concatenate([x[sl].T, h[sl].T], axis=0))  # [K2, BL]
        in_maps.append({**shared,
                        "xh16s": _pack_act(xhT.astype(BFNP), 4),
                        "xh8s": _pack_act(xhT.astype(E4NP), 4),
                        "cTs": _pack_act(
                            np.ascontiguousarray(c[sl].T).astype(BFNP), 2)})
    return in_maps


def _run(inputs, trace=False):
    from concourse.bass_utils import run_bass_kernel_spmd

    if R3_MODE not in _CACHE:
        _CACHE[R3_MODE] = _build(R3_MODE)
    nc = _CACHE[R3_MODE]
    in_maps = _prepare(inputs, R3_MODE)
    res = run_bass_kernel_spmd(nc, in_maps, core_ids=list(range(NCORES)),
                               trace=trace)
    h = np.empty((B, D), np.float32)
    c = np.empty((B, D), np.float32)
    for core in range(NCORES):
        o = res.results[core]["out"]  # [D, 2, BL] bf16
        sl = slice(core * BL, (core + 1) * BL)
        h[sl] = o[:, 0].T.astype(np.float32)
        c[sl] = o[:, 1].T.astype(np.float32)
    return (h, c), res


def kernel(**inputs):
    (h, c), _ = _run(inputs, trace=False)
    return (h, c)


# revision 35
# speedup vs baseline: 1.0220x; 1.0024x over previous
"""AdaptiveLSTMCellWithRes on 8 TRN2 NeuronCores.

Data-parallel over batch (1024 rows/core), weights replicated.
All on-chip compute happens in transposed-activation space [feat, batch].
Mixed precision:
  - i, s, c_hat, a1, r1, r2, r3 matmuls run fp8(e4m3) with DoubleRow
    perf mode (2 k-tiles per PE pass, 2x the bf16 rate). Weights are
    pre-scaled by 64 on host (0.02-std values would land subnormal in
    e4m3); the 1/64 folds into the PSUM-evicting activation's scale.
  - f, o (the error-critical gates: f multiplies c_prev, o multiplies
    tanh(c) directly) run bf16.
  - PSUM, biases and the elementwise combine stay fp32; h/c outputs
    are written bf16 (well inside the error budget, halves store DMA).
Schedule: PE pass count is the floor (~212ns per 512-wide pass), so
everything else hides behind it:
  - Sync queue: the fp8 xh stream first (cold-start critical), then
    in-loop a1/phase-B weight slabs.
  - GpSimd queue: a1's first slab, biases, then ALL r1/r2/r3 fp8 slabs
    prefetched during phase A1 (they were arriving late when loaded
    just-in-time), then the dep-gated bulk bf16 activation loads, the
    alpha DRAM roundtrip, and all output stores (keeps ScalarE free
    for PSUM evictions, which the tail chain is latency-bound on).
  - ScalarE: PSUM evictions only. Dummy Relu/Sigmoid/Tanh activations
    run in the prologue so both ACT_TABLE_LOADs (1.3us each) happen
    before the matmul stream, not in the middle of it.
  - PE warm-up: 4 dummy bf16 matmuls on zeroed scratch keep the HAM
    activity window busy from ~7.3us while the first transfers land;
    real a1 work starts ~9us (vs 14.4us with the old 16-warmup
    prologue) and the clock is at 2.4GHz by ~10.7us.
"""

import os
import sys

if "/opt/trn_rl_repo" not in sys.path:
    sys.path.insert(0, "/opt/trn_rl_repo")

import numpy as np
import ml_dtypes

P = 128
B = 8192          # global batch
NCORES = 8
BL = B // NCORES  # batch per core (1024)
D = 1024          # feature dim
K2 = 2048         # concat(x, h) contraction
JC = D // P       # 8 output-feature tiles
KC2 = K2 // P     # 16 k-chunks for gates/a1
KC1 = D // P      # 8 k-chunks for residual/a2
TC2 = KC2 // 2    # 8 fp8 double-row pair steps
NH = BL // 2      # moving free dim per matmul (512)
WS = 64.0         # fp8 weight pre-scale
QK = 4            # k-chunks per packed activation quarter-tile
NWARM = 8         # PE warm-up matmuls: keep the HAM activity window
                  # busy until the first xh8 quarters land (~10.5us);
                  # j0's passes then carry the activity to full arrival

# r3 precision: "full8" (all 8 k-tiles fp8 DR), "half8" (k 0..3 fp8,
# k 4..7 bf16), "bf16" (all bf16)
R3_MODE = os.environ.get("R3_MODE", "full8")

E4NP = ml_dtypes.float8_e4m3
BFNP = ml_dtypes.bfloat16

_CACHE = {}


def _build(r3_mode):
    import concourse.bass as bass  # noqa: F401
    from concourse import bacc, mybir
    import concourse.tile as tile

    F32 = mybir.dt.float32
    BF16 = mybir.dt.bfloat16
    FP8 = mybir.dt.float8e4
    AF = mybir.ActivationFunctionType
    DR = mybir.MatmulPerfMode.DoubleRow

    nc = bacc.Bacc()

    # fp8 gates (c, s, i, a1): pack[g, j, p, kk, m] = e4m3(WS * Wg[j*128+m, kk*128+p])
    w8 = nc.declare_dram_parameter("w8", [4, JC, P, KC2, P], FP8, isOutput=False)
    # fp8 r1/r2 weights: partition-first pack so the whole matrix loads
    # as ONE fully-contiguous-per-partition DMA
    w8r1 = nc.declare_dram_parameter("w8r1", [P, JC, KC1, P], FP8,
                                     isOutput=False)
    w8r2 = nc.declare_dram_parameter("w8r2", [P, JC, KC1, P], FP8,
                                     isOutput=False)
    if r3_mode == "full8":
        w8r3 = nc.declare_dram_parameter("w8r3", [P, JC, KC1, P], FP8,
                                         isOutput=False)
    elif r3_mode == "half8":
        w8r3 = nc.declare_dram_parameter("w8r3", [P, JC, KC1 // 2, P], FP8,
                                         isOutput=False)
        # bf16 half, pre-scaled by WS so it shares r3's PSUM scale
        wr3h = nc.declare_dram_parameter("wr3h", [JC, P, (KC1 // 2) * P],
                                         BF16, isOutput=False)
    else:
        w8r3 = None
        wr3 = nc.declare_dram_parameter("wr3", [JC, P, D], BF16,
                                        isOutput=False)
    # bf16 gates (f, o): pack[g, j, p, k*128+m] = W[j*128+m, k*128+p]
    wf = nc.declare_dram_parameter("wf", [2, JC, P, K2], BF16, isOutput=False)
    # a2 weight: [P, KC1, 16] e4m3; col 0 holds WS * a2_w[0, k*128+p],
    # cols 1-15 are zero padding (DoubleRow LDWEIGHTS requires the pair
    # step to be a multiple of 16 bytes — s3_lw dual-fp8 restriction)
    a2p = nc.declare_dram_parameter("a2p", [P, KC1, 16], FP8, isOutput=False)
    # biases: [P, 10*JC]; col v*JC+j holds vec_v[j*128:(j+1)*128]
    # v: 0..4 = combined gate biases (i,f,o,c,s), 5=a1_b, 6=r1_b, 7=r2_b,
    # 8=r3_b, 9=a2_b (replicated)
    biasp = nc.declare_dram_parameter("biasp", [P, 10 * JC], F32, isOutput=False)
    # activations pre-swizzled into quarter tiles: [q, p, kk, n] =
    # act[(q*QK+kk)*128+p, n]; q 0..1 = x^T, 2..3 = h^T
    xh16s = nc.declare_dram_parameter("xh16s", [4, P, QK, BL], BF16,
                                      isOutput=False)
    xh8s = nc.declare_dram_parameter("xh8s", [4, P, QK, BL], FP8,
                                     isOutput=False)
    cTs = nc.declare_dram_parameter("cTs", [2, P, QK, BL], BF16, isOutput=False)
    # out[d, 0, n] = h_t^T, out[d, 1, n] = c_t^T (bf16)
    out = nc.declare_dram_parameter("out", [D, 2, BL], BF16, isOutput=True)

    alpha_dram = nc.dram_tensor("alpha_dram", [1, BL], F32)

    with tile.TileContext(nc) as tc:
        with (
            tc.tile_pool(name="consts", bufs=1) as consts,
            tc.tile_pool(name="xh8", bufs=1) as xh8_pool,
            tc.tile_pool(name="xh16", bufs=1) as xh16_pool,
            tc.tile_pool(name="cpre", bufs=1) as cp_pool,
            tc.tile_pool(name="w8p", bufs=5) as w8_pool,
            tc.tile_pool(name="wfp", bufs=4) as wf_pool,
            tc.tile_pool(name="a1s", bufs=4) as a1_pool,
            tc.tile_pool(name="r1", bufs=1) as r1_pool,
            tc.tile_pool(name="r2", bufs=1) as r2_pool,
            tc.tile_pool(name="gates", bufs=1) as g_pool,
            tc.tile_pool(name="ew", bufs=2) as ew_pool,
            tc.tile_pool(name="psum", bufs=3, space="PSUM") as psum_pool,
            tc.tile_pool(name="psum_a2", bufs=1, space="PSUM") as psum_a2_pool,
        ):
            bias_sb = consts.tile([P, 10 * JC], F32, name="bias_sb")
            a2_sb = consts.tile([P, KC1, 16], FP8, name="a2_sb")

            def bias_ap(v, j):
                return bias_sb[:, v * JC + j: v * JC + j + 1]

            # ---- prefix. PE warm-up first: dummy bf16 matmuls on
            # vector-zeroed scratch keep the HAM activity window busy
            # from ~7.3us (right after the framework preamble) while
            # the first transfers land. They write complete start/stop
            # groups into the a2 bank, which the real a2 accumulation
            # later re-zeroes (start=True).
            ps_a2 = [psum_a2_pool.tile([16, NH], F32, tag="a20", name="psa20"),
                     psum_a2_pool.tile([16, NH], F32, tag="a21", name="psa21")]
            scr_s = consts.tile([P, 2], BF16, name="scr_s")
            scr_m = consts.tile([P, NH], BF16, name="scr_m")
            nc.vector.memzero(scr_s[:])
            nc.vector.memzero(scr_m[:])
            for _ in range(NWARM):
                nc.tensor.matmul(ps_a2[0][0:1, :], scr_s[:, 0:1], scr_m[:],
                                 start=True, stop=True)
            # dummy activations: force both ACT_TABLE_LOADs (~1.3us
            # each on ScalarE) into the prologue shadow
            dum = consts.tile([P, 2], F32, name="dum")
            nc.scalar.activation(dum[:], scr_s[:], AF.Relu)
            nc.scalar.activation(dum[:], scr_s[:], AF.Sigmoid)
            nc.scalar.activation(dum[:], scr_s[:], AF.Tanh)

            # Sync queue: the fp8 xh stream, cold-start critical. First
            # quarter split in two so the first matmul's moving data
            # lands sooner on the cold DMA path.
            x8a = xh8_pool.tile([P, 2, BL], FP8, tag="x8a", name="x8a")
            nc.sync.dma_start(out=x8a[:], in_=xh8s[0][:, 0:2, :])
            x8b = xh8_pool.tile([P, 2, BL], FP8, tag="x8b", name="x8b")
            nc.sync.dma_start(out=x8b[:], in_=xh8s[0][:, 2:4, :])
            xh8q = [None]
            for q in range(1, 4):
                tl = xh8_pool.tile([P, QK, BL], FP8, tag=f"x8{q}", name=f"x8{q}")
                nc.sync.dma_start(out=tl[:], in_=xh8s[q])
                xh8q.append(tl)

            # GpSimd queue: a1's first weight slab (needed ~9us) and the
            # small consts. The full r1/r2/r3 matrices follow as ONE
            # contiguous DMA each, dep-gated behind the first a1
            # eviction so they don't steal HBM from the critical x8 /
            # a1-slab streams that pace the start of phase A1.
            a1w0 = w8_pool.tile([P, KC2, P], FP8, tag="w8", name="w8_a1_0")
            nc.gpsimd.dma_start(out=a1w0[:], in_=w8[3, 0])
            nc.gpsimd.dma_start(out=bias_sb[:], in_=biasp[:, :])
            nc.gpsimd.dma_start(out=a2_sb[:], in_=a2p[:, :])
            r1big = consts.tile([P, JC, KC1, P], FP8, name="r1big")
            r2big = consts.tile([P, JC, KC1, P], FP8, name="r2big")
            r3big = None
            if r3_mode == "full8":
                r3big = consts.tile([P, JC, KC1, P], FP8, name="r3big")
            elif r3_mode == "half8":
                r3big = consts.tile([P, JC, KC1 // 2, P], FP8, name="r3big")

            def load_rbigs(dep_ap):
                for big, src in ((r1big, w8r1), (r2big, w8r2),
                                 (r3big, w8r3 if r3_mode != "bf16" else None)):
                    if big is None:
                        continue
                    nc.vector.tensor_copy(big[0:1, 0, 0, 0:1], dep_ap)
                    nc.gpsimd.dma_start(out=big[:], in_=src[:])

            xh16q = [None] * 4
            cpq = []

            def load_bulk_xh16(dep_ap):
                # bulk bf16 loads for phase B on the gpsimd queue. The
                # scheduler orders DMAs by dependency, not program
                # order, so a tiny write sourced from a phase-A product
                # (overwritten by the DMA) holds these transfers back
                # until the critical fp8 prefix stream has landed.
                for q in (0, 1, 2, 3):
                    tl = xh16_pool.tile([P, QK, BL], BF16, tag=f"x{q}",
                                        name=f"x{q}")
                    nc.vector.tensor_copy(tl[0:1, 0, 0:1], dep_ap)
                    nc.gpsimd.dma_start(out=tl[:], in_=xh16s[q])
                    xh16q[q] = tl

            def load_bulk_cp(dep_ap):
                for q in range(2):
                    tl = cp_pool.tile([P, QK, BL], BF16, tag=f"cp{q}",
                                      name=f"cp{q}")
                    nc.vector.tensor_copy(tl[0:1, 0, 0:1], dep_ap)
                    nc.gpsimd.dma_start(out=tl[:], in_=cTs[q])
                    cpq.append(tl)

            def rhs_xh8(t, mv):
                if t == 0:
                    return x8a[:, :, mv]
                if t == 1:
                    return x8b[:, :, mv]
                q, kk = divmod(2 * t, QK)
                return xh8q[q][:, kk:kk + 2, mv]

            def rhs_xh16(k, mv):
                q, kk = divmod(k, QK)
                return xh16q[q][:, kk:kk + 1, mv]

            def load_w8(g, j):
                wt = w8_pool.tile([P, KC2, P], FP8, tag="w8", name=f"w8_{g}_{j}")
                nc.sync.dma_start(out=wt[:], in_=w8[g, j])
                return wt

            def load_wg16(g, j):
                wt = wf_pool.tile([P, K2], BF16, tag="wg", name=f"wg_{g}_{j}",
                                  bufs=3)
                nc.sync.dma_start(out=wt[:], in_=wf[g, j])
                return wt

            def ps_pair(nm):
                return [psum_pool.tile([P, NH], F32, tag="ps0", name=f"{nm}0"),
                        psum_pool.tile([P, NH], F32, tag="ps1", name=f"{nm}1")]

            # All matmul groups run bh-OUTER with the half evicted as
            # soon as its accumulation stops: the PSUM ring is only 3
            # groups deep, and a new group's start-matmul waits on the
            # 3-back group's last eviction — evicting at half-group
            # keeps that wait off the PE (LDWEIGHTS re-loads per half
            # are fully hidden under the 216ns passes).
            def mm8(ps2, wt, tc=TC2, t0=0, evict=None):
                for bh in range(2):
                    mv = slice(bh * NH, (bh + 1) * NH)
                    for t in range(tc):
                        nc.tensor.matmul(
                            ps2[bh][:], wt[:, 2 * t:2 * t + 2, :],
                            rhs_xh8(t0 + t, mv),
                            start=(t == 0), stop=(t == tc - 1),
                            perf_mode=DR)
                    if evict is not None:
                        evict(bh)

            def mm16(ps2, wt, rhs, kc, koff=0, evict=None):
                for bh in range(2):
                    mv = slice(bh * NH, (bh + 1) * NH)
                    for k in range(kc):
                        nc.tensor.matmul(
                            ps2[bh][:], wt[:, k * P:(k + 1) * P],
                            rhs(koff + k, mv),
                            start=(k == 0), stop=(k == kc - 1))
                    if evict is not None:
                        evict(bh)

            # ---- phase A1: a1 (fp8), deferred a2 matmuls. a1 evicts
            # e4m3 into DoubleRow pair tiles (slot = j parity) so the
            # a2 contraction runs DR too: 8 passes instead of 16, and
            # half the non-DR mode switches in the stream. ----
            pend = []

            def flush_a2():
                tp, pair = pend.pop(0)
                for bh in range(2):
                    nc.tensor.matmul(ps_a2[bh][:],
                                     a2_sb[:, 2 * tp:2 * tp + 2, :],
                                     pair[bh][:, :, :], start=(tp == 0),
                                     stop=(tp == JC // 2 - 1), perf_mode=DR)

            cur_pair = None
            for j in range(JC):
                wt = a1w0 if j == 0 else load_w8(3, j)
                ps2 = ps_pair("ps_a1_")
                if j % 2 == 0:
                    cur_pair = [a1_pool.tile([P, 2, NH], FP8, tag=f"a1p{bh}",
                                             name="a1p", bufs=3)
                                for bh in range(2)]

                def ev_a1(bh, j=j, cp=cur_pair, ps2=ps2):
                    nc.scalar.activation(cp[bh][:, j % 2, :], ps2[bh][:],
                                         AF.Relu, bias=bias_ap(5, j),
                                         scale=1.0 / WS)
                if j == 0:
                    # j0 is paced by the arriving xh8 stream: t-outer so
                    # only the last-quarter passes remain when q3 lands.
                    # Filler matmuls on the warm-up scratch plug the
                    # quarter-arrival stalls so the HAM activity window
                    # never re-throttles the clock mid-prologue.
                    FILL = {3: 4, 5: 5}
                    for t in range(TC2):
                        for bh in range(2):
                            mv = slice(bh * NH, (bh + 1) * NH)
                            nc.tensor.matmul(
                                ps2[bh][:], wt[:, 2 * t:2 * t + 2, :],
                                rhs_xh8(t, mv), start=(t == 0),
                                stop=(t == TC2 - 1), perf_mode=DR)
                        for _ in range(FILL.get(t, 0)):
                            nc.tensor.matmul(ps_a2[0][0:1, :], scr_s[:, 0:1],
                                             scr_m[:], start=True, stop=True)
                    ev_a1(0)
                    ev_a1(1)
                else:
                    mm8(ps2, wt, evict=ev_a1)
                if j % 2 == 1:
                    pend.append((j // 2, cur_pair))
                # defer the a2 matmuls one pair so PE never waits on ScalarE
                if len(pend) == 2:
                    flush_a2()
                # staged bulk prefetch, ordered by when each wave is
                # needed; each is dep-gated so HBM stays clear for the
                # stream that paces the current phase (r-bigs must not
                # steal HBM from the a1 slab stream that paces j1-j3)
                if j == 2:
                    load_rbigs(cur_pair[0][0:1, 0, 0:1])
                if j == 4:
                    load_bulk_xh16(cur_pair[0][0:1, 0, 0:1])
                if j == 6:
                    load_bulk_cp(cur_pair[0][0:1, 0, 0:1])
            while pend:
                flush_a2()

            # alpha = sigmoid(a2 @ a1relu + a2_b): [1, BL]; broadcast via
            # DRAM roundtrip that hides under the r1/r2 phases. The 1/WS
            # a2-weight prescale folds into the eviction scale.
            for bh in range(2):
                asb = a1_pool.tile([1, NH], F32, tag="asb", name="alpha_sb")
                nc.scalar.activation(asb[:], ps_a2[bh][0:1, :], AF.Sigmoid,
                                     bias=bias_sb[0:1, 9 * JC: 9 * JC + 1],
                                     scale=1.0 / WS)
                nc.gpsimd.dma_start(
                    out=alpha_dram[0:1, bh * NH:(bh + 1) * NH], in_=asb[:])
            alpha_rep = consts.tile([P, BL], F32, name="alpha_rep")
            nc.gpsimd.dma_start(
                out=alpha_rep[:], in_=alpha_dram[0:1, :].broadcast_to([P, BL]))

            # ---- phase A2: r1 = relu(h @ r1_w.T + b) in fp8 (moving = the
            # resident fp8 h-half). r1 evicts straight to e4m3 pair-tiles
            # so r2 can also run fp8 DoubleRow. ----
            r1q = [r1_pool.tile([P, QK, BL], FP8, tag="r1a", name="r1a"),
                   r1_pool.tile([P, QK, BL], FP8, tag="r1b", name="r1b")]
            for j in range(JC):
                ps2 = ps_pair("ps_r1_")
                qq, jj = divmod(j, QK)
                for bh in range(2):
                    mv = slice(bh * NH, (bh + 1) * NH)
                    for t in range(KC1 // 2):
                        nc.tensor.matmul(
                            ps2[bh][:], r1big[:, j, 2 * t:2 * t + 2, :],
                            rhs_xh8(TC2 // 2 + t, mv),
                            start=(t == 0), stop=(t == KC1 // 2 - 1),
                            perf_mode=DR)
                    nc.scalar.activation(
                        r1q[qq][:, jj, mv],
                        ps2[bh][:], AF.Relu, bias=bias_ap(6, j),
                        scale=1.0 / WS)

            # ---- phase A3: r2 = relu(r1 @ r2_w.T + b) in fp8 DoubleRow.
            # Eviction dtype depends on r3's precision: e4m3 quarter
            # tiles where r3 runs DoubleRow, bf16 where it runs bf16.
            n8 = {"full8": JC, "half8": QK, "bf16": 0}[r3_mode]
            r2q8 = []
            if n8:
                r2q8 = [r1_pool.tile([P, QK, BL], FP8, tag="r2qa", name="r2qa")]
                if n8 > QK:
                    r2q8.append(
                        r1_pool.tile([P, QK, BL], FP8, tag="r2qb", name="r2qb"))
            r2 = []
            for j in range(JC):
                ps2 = ps_pair("ps_r2_")
                qq, jj = divmod(j, QK)
                t_ = None
                if j >= n8:
                    t_ = r2_pool.tile([P, BL], BF16, tag=f"r2_{j}",
                                      name=f"r2_{j}")
                    r2.append(t_)
                for bh in range(2):
                    mv = slice(bh * NH, (bh + 1) * NH)
                    for t in range(KC1 // 2):
                        nc.tensor.matmul(
                            ps2[bh][:], r2big[:, j, 2 * t:2 * t + 2, :],
                            r1q[t // 2][:, (2 * t) % QK:(2 * t) % QK + 2, mv],
                            start=(t == 0), stop=(t == KC1 // 2 - 1),
                            perf_mode=DR)
                    dst = (r2q8[qq][:, jj, mv] if j < n8 else t_[:, mv])
                    nc.scalar.activation(dst, ps2[bh][:], AF.Relu,
                                         bias=bias_ap(7, j), scale=1.0 / WS)

            # ---- phase B: gates + r3 + combine, per feature tile j.
            # Order c,s,i (fp8), o, f (bf16), r3: the elementwise chain runs
            # while later matmuls stream; o comes before f/r3 so only the
            # short r3-evict -> add -> tanh -> mul chain trails the last MM.
            GATE8 = {"c": (0, 3, AF.Tanh), "s": (1, 4, AF.Sigmoid),
                     "i": (2, 0, AF.Sigmoid)}

            def gate8(key, j):
                gi, v, fn = GATE8[key]
                wt = load_w8(gi, j)
                t_ = g_pool.tile([P, BL], F32, tag=f"g8{key}", name=f"g8{key}")
                ps2 = ps_pair("ps_g8")

                def ev(bh):
                    mv = slice(bh * NH, (bh + 1) * NH)
                    nc.scalar.activation(t_[:, mv], ps2[bh][:], fn,
                                         bias=bias_ap(v, j), scale=1.0 / WS)
                mm8(ps2, wt, evict=ev)
                return t_

            def gate16(gi, v, j):
                wt = load_wg16(gi, j)
                t_ = g_pool.tile([P, BL], F32, tag=f"g16{gi}", name=f"g16{gi}")
                ps2 = ps_pair("ps_g16")

                def ev(bh):
                    mv = slice(bh * NH, (bh + 1) * NH)
                    nc.scalar.activation(t_[:, mv], ps2[bh][:], AF.Sigmoid,
                                         bias=bias_ap(v, j))
                mm16(ps2, wt, rhs_xh16, KC2, evict=ev)
                return t_

            def rhs_r2q8(t, mv):
                # DR pair t over the fp8 r2 quarters
                return r2q8[t // 2][:, (2 * t) % QK:(2 * t) % QK + 2, mv]

            def rhs_r2_16(k, mv):
                # bf16 r2 tiles; in half8 mode tile list starts at k=QK
                return r2[k - (QK if r3_mode == "half8" else 0)][:, mv]

            def mm_r3(ps2, j, wt3b, evict):
                # r3 accumulation: fp8 DR pairs then bf16 k-tiles (both
                # weight halves pre-scaled by WS so PSUM shares one scale)
                n_dr = {"full8": KC1 // 2, "half8": KC1 // 4, "bf16": 0}[r3_mode]
                n_16 = KC1 - 2 * n_dr
                for bh in range(2):
                    mv = slice(bh * NH, (bh + 1) * NH)
                    for t in range(n_dr):
                        nc.tensor.matmul(
                            ps2[bh][:], r3big[:, j, 2 * t:2 * t + 2, :],
                            rhs_r2q8(t, mv), start=(t == 0),
                            stop=(t == n_dr - 1 and n_16 == 0),
                            perf_mode=DR)
                    for k in range(n_16):
                        nc.tensor.matmul(
                            ps2[bh][:], wt3b[:, k * P:(k + 1) * P],
                            rhs_r2_16(2 * n_dr + k, mv),
                            start=(k == 0 and n_dr == 0),
                            stop=(k == n_16 - 1))
                    evict(bh)

            for j in range(JC):
                last = j == JC - 1
                ch = gate8("c", j)
                st = gate8("s", j)
                it = gate8("i", j)

                t1s = []
                for bh in range(2):
                    mv = slice(bh * NH, (bh + 1) * NH)
                    t1 = ew_pool.tile([P, NH], F32, tag=f"t1{bh}", name="t1")
                    nc.vector.tensor_mul(t1[:], it[:, mv], ch[:, mv])
                    nc.vector.tensor_mul(t1[:], t1[:], st[:, mv])
                    nc.vector.tensor_mul(t1[:], t1[:], alpha_rep[:, mv])
                    t1s.append(t1)

                # last j: f before o, so f's evict -> t2 -> t1 chain
                # hides under o's and r3's matmuls and only the short
                # r3-evict -> add -> tanh -> mul chain trails the last MM
                if last:
                    ft = gate16(0, 1, j)
                    ot = gate16(1, 2, j)
                else:
                    ot = gate16(1, 2, j)
                    ft = gate16(0, 1, j)
                for bh in range(2):
                    mv = slice(bh * NH, (bh + 1) * NH)
                    qq, kk = divmod(j, QK)
                    t2 = ew_pool.tile([P, NH], F32, tag=f"t2{bh}", name="t2",
                                      bufs=1)
                    nc.vector.tensor_mul(t2[:], ft[:, mv],
                                         cpq[qq][:, kk, mv])
                    nc.vector.tensor_add(t1s[bh][:], t1s[bh][:], t2[:])

                if r3_mode == "bf16":
                    wt3b = wf_pool.tile([P, D], BF16, tag="wr", bufs=3,
                                        name=f"wr3_{j}")
                    nc.sync.dma_start(out=wt3b[:], in_=wr3[j])
                elif r3_mode == "half8":
                    wt3b = wf_pool.tile([P, (KC1 // 2) * P], BF16, tag="wr",
                                        bufs=3, name=f"wr3h_{j}")
                    nc.sync.dma_start(out=wt3b[:], in_=wr3h[j])
                else:
                    wt3b = None
                ps2 = ps_pair("ps_r3_")
                # stage[:, 0, :] = h, stage[:, 1, :] = c -> single store.
                # The finish chain runs per batch half right after that
                # half's accumulation stops (overlapping the other
                # half's matmuls); the last j additionally splits into
                # NH/2 chunks to pipeline evict -> add -> tanh -> mul.
                r3scale = 1.0 if r3_mode == "bf16" else 1.0 / WS
                nch = 1 if not last else 2
                hw_ = NH // nch

                def finish(bh):
                    for cc in range(nch):
                        pv = slice(cc * hw_, (cc + 1) * hw_)
                        mv = slice(bh * NH + cc * hw_,
                                   bh * NH + (cc + 1) * hw_)
                        stg = ew_pool.tile([P, 2, hw_], BF16,
                                           tag=f"st{bh}_{nch}{cc}", name="stg")
                        if r3_mode == "bf16":
                            # c = (r3_psum + r3_bias) + t1 in one DVE op
                            # straight from PSUM
                            nc.vector.scalar_tensor_tensor(
                                stg[:, 1, :], ps2[bh][:, pv], bias_ap(8, j),
                                t1s[bh][:, pv],
                                mybir.AluOpType.add, mybir.AluOpType.add)
                        else:
                            # fp8 r3 PSUM carries the WS weight scale:
                            # evict via ScalarE (scale+bias), add on DVE
                            r3sb = ew_pool.tile([P, hw_],
                                                F32 if r3_mode == "full8"
                                                else BF16,
                                                tag=f"r3{bh}{cc}", name="r3sb",
                                                bufs=1)
                            nc.scalar.activation(r3sb[:], ps2[bh][:, pv],
                                                 AF.Identity,
                                                 bias=bias_ap(8, j),
                                                 scale=r3scale)
                            nc.vector.tensor_add(stg[:, 1, :], t1s[bh][:, pv],
                                                 r3sb[:])
                        th = ew_pool.tile([P, hw_],
                                          F32 if r3_mode == "full8" else BF16,
                                          tag=f"th{bh}{cc}",
                                          name="th", bufs=1)
                        nc.scalar.activation(th[:], stg[:, 1, :], AF.Tanh)
                        nc.vector.tensor_mul(stg[:, 0, :], ot[:, mv], th[:])
                        # last j's stores ride Sync (idle by then): a
                        # trailing gpsimd store chain made the epilogue
                        # queue-drain ~5us long
                        eng = nc.sync if last else nc.gpsimd
                        eng.dma_start(
                            out=out[j * P:(j + 1) * P, :, mv], in_=stg[:])
                mm_r3(ps2, j, wt3b, evict=finish)

    nc.finalize()
    return nc


def _pack_w(W, kdim):
    # pack[j, p, k*128+m] = W[j*128+m, k*128+p]
    kc = kdim // P
    return np.ascontiguousarray(
        np.asarray(W, np.float32).reshape(JC, P, kc, P)
        .transpose(0, 3, 2, 1).reshape(JC, P, kc * P))


def _pack_act(aT, nq, qk=QK):
    # aT: [nq*qk*P, BL] -> [nq, P, qk, BL] with [q, p, kk, n] = aT[(q*qk+kk)*P+p, n]
    return np.ascontiguousarray(
        aT.reshape(nq, qk, P, BL).transpose(0, 2, 1, 3))


def _prepare(inputs, r3_mode):
    f = lambda name: np.asarray(inputs[name], dtype=np.float32)

    def comb(g):
        u = "U" + g[1]
        return np.concatenate([f(g + "_w"), f(u + "_w")], axis=1)

    # fp8 gates: c, s, i, a1 (order matches in-kernel GATE8/a1 indices)
    w8 = np.stack([
        _pack_w(comb("Wc") * WS, K2),
        _pack_w(comb("Ws") * WS, K2),
        _pack_w(comb("Wi") * WS, K2),
        _pack_w(f("a1_w") * WS, K2),
    ]).astype(E4NP).reshape(4, JC, P, KC2, P)
    def pack_pfirst(W):
        # [P, JC, KC1, P]: partition dim first so the whole matrix is
        # one contiguous-per-partition DMA
        return np.ascontiguousarray(
            _pack_w(W, D).reshape(JC, P, KC1, P).transpose(1, 0, 2, 3))

    w8r1 = pack_pfirst(f("r1_w") * WS).astype(E4NP)
    w8r2 = pack_pfirst(f("r2_w") * WS).astype(E4NP)
    # bf16 gates: f, o
    wf_ = np.stack([_pack_w(comb("Wf"), K2),
                    _pack_w(comb("Wo"), K2)]).astype(BFNP)
    a2p = np.zeros((P, KC1, 16), np.float32)
    a2p[:, :, 0] = (f("a2_w") * WS).reshape(KC1, P).T
    a2p = a2p.astype(E4NP)

    shared = {"w8": w8, "w8r1": w8r1, "w8r2": w8r2, "wf": wf_, "a2p": a2p}
    r3pf = pack_pfirst(f("r3_w") * WS)  # [P, JC, KC1, P], scaled by WS
    if r3_mode == "full8":
        shared["w8r3"] = r3pf.astype(E4NP)
    elif r3_mode == "half8":
        shared["w8r3"] = np.ascontiguousarray(
            r3pf[:, :, :KC1 // 2]).astype(E4NP)
        shared["wr3h"] = np.ascontiguousarray(
            r3pf.transpose(1, 0, 2, 3)[:, :, KC1 // 2:]
            .reshape(JC, P, (KC1 // 2) * P)).astype(BFNP)
    else:
        shared["wr3"] = _pack_w(f("r3_w"), D).astype(BFNP)

    bias_vecs = []
    for g in ("Wi", "Wf", "Wo", "Wc", "Ws"):
        u = "U" + g[1]
        bias_vecs.append(f(g + "_b") + f(u + "_b"))
    bias_vecs += [f("a1_b"), f("r1_b"), f("r2_b"), f("r3_b"),
                  np.full(D, f("a2_b")[0], np.float32)]
    # biasp[p, v*JC + j] = vec_v[j*128 + p]
    biasp = np.ascontiguousarray(
        np.stack(bias_vecs).reshape(10, JC, P).transpose(2, 0, 1)
        .reshape(P, 10 * JC))
    shared["biasp"] = biasp

    x, h, c = f("x"), f("h_prev"), f("c_prev")
    in_maps = []
    for core in range(NCORES):
        sl = slice(core * BL, (core + 1) * BL)
        xhT = np.ascontiguousarray(
            np.concatenate([x[sl].T, h[sl].T], axis=0))  # [K2, BL]
        in_maps.append({**shared,
                        "xh16s": _pack_act(xhT.astype(BFNP), 4),
                        "xh8s": _pack_act(xhT.astype(E4NP), 4),
                        "cTs": _pack_act(
                            np.ascontiguousarray(c[sl].T).astype(BFNP), 2)})
    return in_maps


def _run(inputs, trace=False):
    from concourse.bass_utils import run_bass_kernel_spmd

    if R3_MODE not in _CACHE:
        _CACHE[R3_MODE] = _build(R3_MODE)
    nc = _CACHE[R3_MODE]
    in_maps = _prepare(inputs, R3_MODE)
    res = run_bass_kernel_spmd(nc, in_maps, core_ids=list(range(NCORES)),
                               trace=trace)
    h = np.empty((B, D), np.float32)
    c = np.empty((B, D), np.float32)
    for core in range(NCORES):
        o = res.results[core]["out"]  # [D, 2, BL] bf16
        sl = slice(core * BL, (core + 1) * BL)
        h[sl] = o[:, 0].T.astype(np.float32)
        c[sl] = o[:, 1].T.astype(np.float32)
    return (h, c), res


def kernel(**inputs):
    (h, c), _ = _run(inputs, trace=False)
    return (h, c)


# revision 36
# speedup vs baseline: 1.0308x; 1.0086x over previous
"""AdaptiveLSTMCellWithRes on 8 TRN2 NeuronCores.

Data-parallel over batch (1024 rows/core), weights replicated.
All on-chip compute happens in transposed-activation space [feat, batch].
Mixed precision:
  - i, s, c_hat, a1, r1, r2, r3 matmuls run fp8(e4m3) with DoubleRow
    perf mode (2 k-tiles per PE pass, 2x the bf16 rate). Weights are
    pre-scaled by 64 on host (0.02-std values would land subnormal in
    e4m3); the 1/64 folds into the PSUM-evicting activation's scale.
  - f, o (the error-critical gates: f multiplies c_prev, o multiplies
    tanh(c) directly) run bf16.
  - PSUM, biases and the elementwise combine stay fp32; h/c outputs
    are written bf16 (well inside the error budget, halves store DMA).
Schedule: PE pass count is the floor (~212ns per 512-wide pass), so
everything else hides behind it:
  - Sync queue: the fp8 xh stream first (cold-start critical), then
    in-loop a1/phase-B weight slabs.
  - GpSimd queue: a1's first slab, biases, then ALL r1/r2/r3 fp8 slabs
    prefetched during phase A1 (they were arriving late when loaded
    just-in-time), then the dep-gated bulk bf16 activation loads, the
    alpha DRAM roundtrip, and all output stores (keeps ScalarE free
    for PSUM evictions, which the tail chain is latency-bound on).
  - ScalarE: PSUM evictions only. Dummy Relu/Sigmoid/Tanh activations
    run in the prologue so both ACT_TABLE_LOADs (1.3us each) happen
    before the matmul stream, not in the middle of it.
  - PE warm-up: 4 dummy bf16 matmuls on zeroed scratch keep the HAM
    activity window busy from ~7.3us while the first transfers land;
    real a1 work starts ~9us (vs 14.4us with the old 16-warmup
    prologue) and the clock is at 2.4GHz by ~10.7us.
"""

import os
import sys

if "/opt/trn_rl_repo" not in sys.path:
    sys.path.insert(0, "/opt/trn_rl_repo")

import numpy as np
import ml_dtypes

P = 128
B = 8192          # global batch
NCORES = 8
BL = B // NCORES  # batch per core (1024)
D = 1024          # feature dim
K2 = 2048         # concat(x, h) contraction
JC = D // P       # 8 output-feature tiles
KC2 = K2 // P     # 16 k-chunks for gates/a1
KC1 = D // P      # 8 k-chunks for residual/a2
TC2 = KC2 // 2    # 8 fp8 double-row pair steps
NH = BL // 2      # moving free dim per matmul (512)
WS = 64.0         # fp8 weight pre-scale
QK = 4            # k-chunks per packed activation quarter-tile
NWARM = 12        # PE warm-up matmuls: keep the HAM activity window
                  # busy until the early xh8 quarters land (~12.2us);
                  # j0's passes then carry the activity to full arrival

# r3 precision: "full8" (all 8 k-tiles fp8 DR), "half8" (k 0..3 fp8,
# k 4..7 bf16), "bf16" (all bf16)
R3_MODE = os.environ.get("R3_MODE", "full8")

E4NP = ml_dtypes.float8_e4m3
BFNP = ml_dtypes.bfloat16

_CACHE = {}


def _build(r3_mode):
    import concourse.bass as bass  # noqa: F401
    from concourse import bacc, mybir
    import concourse.tile as tile

    F32 = mybir.dt.float32
    BF16 = mybir.dt.bfloat16
    FP8 = mybir.dt.float8e4
    AF = mybir.ActivationFunctionType
    DR = mybir.MatmulPerfMode.DoubleRow

    nc = bacc.Bacc()

    # fp8 gates (c, s, i, a1): pack[g, j, p, kk, m] = e4m3(WS * Wg[j*128+m, kk*128+p])
    w8 = nc.declare_dram_parameter("w8", [4, JC, P, KC2, P], FP8, isOutput=False)
    # fp8 r1/r2 weights: partition-first pack so the whole matrix loads
    # as ONE fully-contiguous-per-partition DMA
    w8r1 = nc.declare_dram_parameter("w8r1", [P, JC, KC1, P], FP8,
                                     isOutput=False)
    w8r2 = nc.declare_dram_parameter("w8r2", [P, JC, KC1, P], FP8,
                                     isOutput=False)
    if r3_mode == "full8":
        w8r3 = nc.declare_dram_parameter("w8r3", [P, JC, KC1, P], FP8,
                                         isOutput=False)
    elif r3_mode == "half8":
        w8r3 = nc.declare_dram_parameter("w8r3", [P, JC, KC1 // 2, P], FP8,
                                         isOutput=False)
        # bf16 half, pre-scaled by WS so it shares r3's PSUM scale
        wr3h = nc.declare_dram_parameter("wr3h", [JC, P, (KC1 // 2) * P],
                                         BF16, isOutput=False)
    else:
        w8r3 = None
        wr3 = nc.declare_dram_parameter("wr3", [JC, P, D], BF16,
                                        isOutput=False)
    # bf16 gates (f, o): pack[g, j, p, k*128+m] = W[j*128+m, k*128+p]
    wf = nc.declare_dram_parameter("wf", [2, JC, P, K2], BF16, isOutput=False)
    # a2 weight: [P, KC1, 16] e4m3; col 0 holds WS * a2_w[0, k*128+p],
    # cols 1-15 are zero padding (DoubleRow LDWEIGHTS requires the pair
    # step to be a multiple of 16 bytes — s3_lw dual-fp8 restriction)
    a2p = nc.declare_dram_parameter("a2p", [P, KC1, 16], FP8, isOutput=False)
    # biases: [P, 10*JC]; col v*JC+j holds vec_v[j*128:(j+1)*128]
    # v: 0..4 = combined gate biases (i,f,o,c,s), 5=a1_b, 6=r1_b, 7=r2_b,
    # 8=r3_b, 9=a2_b (replicated)
    biasp = nc.declare_dram_parameter("biasp", [P, 10 * JC], F32, isOutput=False)
    # activations pre-swizzled into quarter tiles: [q, p, kk, n] =
    # act[(q*QK+kk)*128+p, n]; q 0..1 = x^T, 2..3 = h^T
    xh16s = nc.declare_dram_parameter("xh16s", [4, P, QK, BL], BF16,
                                      isOutput=False)
    xh8s = nc.declare_dram_parameter("xh8s", [4, P, QK, BL], FP8,
                                     isOutput=False)
    cTs = nc.declare_dram_parameter("cTs", [2, P, QK, BL], BF16, isOutput=False)
    # out[d, 0, n] = h_t^T, out[d, 1, n] = c_t^T (bf16)
    out = nc.declare_dram_parameter("out", [D, 2, BL], BF16, isOutput=True)

    alpha_dram = nc.dram_tensor("alpha_dram", [1, BL], F32)

    with tile.TileContext(nc) as tc:
        with (
            tc.tile_pool(name="consts", bufs=1) as consts,
            tc.tile_pool(name="xh8", bufs=1) as xh8_pool,
            tc.tile_pool(name="xh16", bufs=1) as xh16_pool,
            tc.tile_pool(name="cpre", bufs=1) as cp_pool,
            tc.tile_pool(name="w8p", bufs=5) as w8_pool,
            tc.tile_pool(name="wfp", bufs=4) as wf_pool,
            tc.tile_pool(name="a1s", bufs=4) as a1_pool,
            tc.tile_pool(name="r1", bufs=1) as r1_pool,
            tc.tile_pool(name="r2", bufs=1) as r2_pool,
            tc.tile_pool(name="gates", bufs=1) as g_pool,
            tc.tile_pool(name="ew", bufs=2) as ew_pool,
            tc.tile_pool(name="psum", bufs=3, space="PSUM") as psum_pool,
            tc.tile_pool(name="psum_a2", bufs=1, space="PSUM") as psum_a2_pool,
        ):
            bias_sb = consts.tile([P, 10 * JC], F32, name="bias_sb")
            a2_sb = consts.tile([P, KC1, 16], FP8, name="a2_sb")

            def bias_ap(v, j):
                return bias_sb[:, v * JC + j: v * JC + j + 1]

            # ---- prefix. PE warm-up first: dummy bf16 matmuls on
            # vector-zeroed scratch keep the HAM activity window busy
            # from ~7.3us (right after the framework preamble) while
            # the first transfers land. They write complete start/stop
            # groups into the a2 bank, which the real a2 accumulation
            # later re-zeroes (start=True).
            ps_a2 = [psum_a2_pool.tile([16, NH], F32, tag="a20", name="psa20"),
                     psum_a2_pool.tile([16, NH], F32, tag="a21", name="psa21")]
            scr_s = consts.tile([P, 2], BF16, name="scr_s")
            scr_m = consts.tile([P, NH], BF16, name="scr_m")
            nc.vector.memzero(scr_s[:])
            nc.vector.memzero(scr_m[:])
            for _ in range(NWARM):
                nc.tensor.matmul(ps_a2[0][0:1, :], scr_s[:, 0:1], scr_m[:],
                                 start=True, stop=True)
            # dummy activations: force both ACT_TABLE_LOADs (~1.3us
            # each on ScalarE) into the prologue shadow
            dum = consts.tile([P, 2], F32, name="dum")
            nc.scalar.activation(dum[:], scr_s[:], AF.Relu)
            nc.scalar.activation(dum[:], scr_s[:], AF.Sigmoid)
            nc.scalar.activation(dum[:], scr_s[:], AF.Tanh)

            # Sync queue: the fp8 xh stream, cold-start critical. First
            # quarter split in two so the first matmul's moving data
            # lands sooner on the cold DMA path.
            x8a = xh8_pool.tile([P, 2, BL], FP8, tag="x8a", name="x8a")
            nc.sync.dma_start(out=x8a[:], in_=xh8s[0][:, 0:2, :])
            x8b = xh8_pool.tile([P, 2, BL], FP8, tag="x8b", name="x8b")
            nc.sync.dma_start(out=x8b[:], in_=xh8s[0][:, 2:4, :])
            xh8q = [None]
            for q in range(1, 4):
                tl = xh8_pool.tile([P, QK, BL], FP8, tag=f"x8{q}", name=f"x8{q}")
                nc.sync.dma_start(out=tl[:], in_=xh8s[q])
                xh8q.append(tl)

            # GpSimd queue: a1's first weight slab (needed ~9us) and the
            # small consts. The full r1/r2/r3 matrices follow as ONE
            # contiguous DMA each, dep-gated behind the first a1
            # eviction so they don't steal HBM from the critical x8 /
            # a1-slab streams that pace the start of phase A1.
            a1w0 = w8_pool.tile([P, KC2, P], FP8, tag="w8", name="w8_a1_0")
            nc.gpsimd.dma_start(out=a1w0[:], in_=w8[3, 0])
            nc.gpsimd.dma_start(out=bias_sb[:], in_=biasp[:, :])
            nc.gpsimd.dma_start(out=a2_sb[:], in_=a2p[:, :])
            r1big = consts.tile([P, JC, KC1, P], FP8, name="r1big")
            r2big = consts.tile([P, JC, KC1, P], FP8, name="r2big")
            r3big = None
            if r3_mode == "full8":
                r3big = consts.tile([P, JC, KC1, P], FP8, name="r3big")
            elif r3_mode == "half8":
                r3big = consts.tile([P, JC, KC1 // 2, P], FP8, name="r3big")

            def load_rbigs(dep_ap):
                for big, src in ((r1big, w8r1), (r2big, w8r2),
                                 (r3big, w8r3 if r3_mode != "bf16" else None)):
                    if big is None:
                        continue
                    nc.vector.tensor_copy(big[0:1, 0, 0, 0:1], dep_ap)
                    nc.gpsimd.dma_start(out=big[:], in_=src[:])

            xh16q = [None] * 4
            cpq = []

            def load_bulk_xh16(dep_ap):
                # bulk bf16 loads for phase B on the gpsimd queue. The
                # scheduler orders DMAs by dependency, not program
                # order, so a tiny write sourced from a phase-A product
                # (overwritten by the DMA) holds these transfers back
                # until the critical fp8 prefix stream has landed.
                for q in (0, 1, 2, 3):
                    tl = xh16_pool.tile([P, QK, BL], BF16, tag=f"x{q}",
                                        name=f"x{q}")
                    nc.vector.tensor_copy(tl[0:1, 0, 0:1], dep_ap)
                    nc.gpsimd.dma_start(out=tl[:], in_=xh16s[q])
                    xh16q[q] = tl

            def load_bulk_cp(dep_ap):
                for q in range(2):
                    tl = cp_pool.tile([P, QK, BL], BF16, tag=f"cp{q}",
                                      name=f"cp{q}")
                    nc.vector.tensor_copy(tl[0:1, 0, 0:1], dep_ap)
                    nc.gpsimd.dma_start(out=tl[:], in_=cTs[q])
                    cpq.append(tl)

            def rhs_xh8(t, mv):
                if t == 0:
                    return x8a[:, :, mv]
                if t == 1:
                    return x8b[:, :, mv]
                q, kk = divmod(2 * t, QK)
                return xh8q[q][:, kk:kk + 2, mv]

            def rhs_xh16(k, mv):
                q, kk = divmod(k, QK)
                return xh16q[q][:, kk:kk + 1, mv]

            def load_w8(g, j):
                wt = w8_pool.tile([P, KC2, P], FP8, tag="w8", name=f"w8_{g}_{j}")
                nc.sync.dma_start(out=wt[:], in_=w8[g, j])
                return wt

            def load_wg16(g, j):
                wt = wf_pool.tile([P, K2], BF16, tag="wg", name=f"wg_{g}_{j}",
                                  bufs=3)
                nc.sync.dma_start(out=wt[:], in_=wf[g, j])
                return wt

            def ps_pair(nm):
                return [psum_pool.tile([P, NH], F32, tag="ps0", name=f"{nm}0"),
                        psum_pool.tile([P, NH], F32, tag="ps1", name=f"{nm}1")]

            # All matmul groups run bh-OUTER with the half evicted as
            # soon as its accumulation stops: the PSUM ring is only 3
            # groups deep, and a new group's start-matmul waits on the
            # 3-back group's last eviction — evicting at half-group
            # keeps that wait off the PE (LDWEIGHTS re-loads per half
            # are fully hidden under the 216ns passes).
            def mm8(ps2, wt, tc=TC2, t0=0, evict=None):
                for bh in range(2):
                    mv = slice(bh * NH, (bh + 1) * NH)
                    for t in range(tc):
                        nc.tensor.matmul(
                            ps2[bh][:], wt[:, 2 * t:2 * t + 2, :],
                            rhs_xh8(t0 + t, mv),
                            start=(t == 0), stop=(t == tc - 1),
                            perf_mode=DR)
                    if evict is not None:
                        evict(bh)

            def mm16(ps2, wt, rhs, kc, koff=0, evict=None):
                for bh in range(2):
                    mv = slice(bh * NH, (bh + 1) * NH)
                    for k in range(kc):
                        nc.tensor.matmul(
                            ps2[bh][:], wt[:, k * P:(k + 1) * P],
                            rhs(koff + k, mv),
                            start=(k == 0), stop=(k == kc - 1))
                    if evict is not None:
                        evict(bh)

            # ---- phase A1: a1 (fp8), deferred a2 matmuls. a1 evicts
            # e4m3 into DoubleRow pair tiles (slot = j parity) so the
            # a2 contraction runs DR too: 8 passes instead of 16, and
            # half the non-DR mode switches in the stream. ----
            pend = []

            def flush_a2():
                tp, pair = pend.pop(0)
                for bh in range(2):
                    nc.tensor.matmul(ps_a2[bh][:],
                                     a2_sb[:, 2 * tp:2 * tp + 2, :],
                                     pair[bh][:, :, :], start=(tp == 0),
                                     stop=(tp == JC // 2 - 1), perf_mode=DR)

            cur_pair = None
            for j in range(JC):
                wt = a1w0 if j == 0 else load_w8(3, j)
                ps2 = ps_pair("ps_a1_")
                if j % 2 == 0:
                    cur_pair = [a1_pool.tile([P, 2, NH], FP8, tag=f"a1p{bh}",
                                             name="a1p", bufs=3)
                                for bh in range(2)]

                def ev_a1(bh, j=j, cp=cur_pair, ps2=ps2):
                    nc.scalar.activation(cp[bh][:, j % 2, :], ps2[bh][:],
                                         AF.Relu, bias=bias_ap(5, j),
                                         scale=1.0 / WS)
                if j == 0:
                    # j0 is paced by the arriving xh8 stream: t-outer so
                    # only the last-quarter passes remain when q3 lands
                    for t in range(TC2):
                        for bh in range(2):
                            mv = slice(bh * NH, (bh + 1) * NH)
                            nc.tensor.matmul(
                                ps2[bh][:], wt[:, 2 * t:2 * t + 2, :],
                                rhs_xh8(t, mv), start=(t == 0),
                                stop=(t == TC2 - 1), perf_mode=DR)
                    ev_a1(0)
                    ev_a1(1)
                else:
                    mm8(ps2, wt, evict=ev_a1)
                if j % 2 == 1:
                    pend.append((j // 2, cur_pair))
                # defer the a2 matmuls one pair so PE never waits on ScalarE
                if len(pend) == 2:
                    flush_a2()
                # staged bulk prefetch, ordered by when each wave is
                # needed; each is dep-gated so HBM stays clear for the
                # stream that paces the current phase (r-bigs must not
                # steal HBM from the a1 slab stream that paces j1-j3)
                if j == 2:
                    load_rbigs(cur_pair[0][0:1, 0, 0:1])
                if j == 4:
                    load_bulk_xh16(cur_pair[0][0:1, 0, 0:1])
                if j == 6:
                    load_bulk_cp(cur_pair[0][0:1, 0, 0:1])
            while pend:
                flush_a2()

            # alpha = sigmoid(a2 @ a1relu + a2_b): [1, BL]; broadcast via
            # DRAM roundtrip that hides under the r1/r2 phases. The 1/WS
            # a2-weight prescale folds into the eviction scale.
            for bh in range(2):
                asb = a1_pool.tile([1, NH], F32, tag="asb", name="alpha_sb")
                nc.scalar.activation(asb[:], ps_a2[bh][0:1, :], AF.Sigmoid,
                                     bias=bias_sb[0:1, 9 * JC: 9 * JC + 1],
                                     scale=1.0 / WS)
                nc.gpsimd.dma_start(
                    out=alpha_dram[0:1, bh * NH:(bh + 1) * NH], in_=asb[:])
            alpha_rep = consts.tile([P, BL], F32, name="alpha_rep")
            nc.gpsimd.dma_start(
                out=alpha_rep[:], in_=alpha_dram[0:1, :].broadcast_to([P, BL]))

            # ---- phase A2: r1 = relu(h @ r1_w.T + b) in fp8 (moving = the
            # resident fp8 h-half). r1 evicts straight to e4m3 pair-tiles
            # so r2 can also run fp8 DoubleRow. ----
            r1q = [r1_pool.tile([P, QK, BL], FP8, tag="r1a", name="r1a"),
                   r1_pool.tile([P, QK, BL], FP8, tag="r1b", name="r1b")]
            for j in range(JC):
                ps2 = ps_pair("ps_r1_")
                qq, jj = divmod(j, QK)
                for bh in range(2):
                    mv = slice(bh * NH, (bh + 1) * NH)
                    for t in range(KC1 // 2):
                        nc.tensor.matmul(
                            ps2[bh][:], r1big[:, j, 2 * t:2 * t + 2, :],
                            rhs_xh8(TC2 // 2 + t, mv),
                            start=(t == 0), stop=(t == KC1 // 2 - 1),
                            perf_mode=DR)
                    nc.scalar.activation(
                        r1q[qq][:, jj, mv],
                        ps2[bh][:], AF.Relu, bias=bias_ap(6, j),
                        scale=1.0 / WS)

            # ---- phase A3: r2 = relu(r1 @ r2_w.T + b) in fp8 DoubleRow.
            # Eviction dtype depends on r3's precision: e4m3 quarter
            # tiles where r3 runs DoubleRow, bf16 where it runs bf16.
            n8 = {"full8": JC, "half8": QK, "bf16": 0}[r3_mode]
            r2q8 = []
            if n8:
                r2q8 = [r1_pool.tile([P, QK, BL], FP8, tag="r2qa", name="r2qa")]
                if n8 > QK:
                    r2q8.append(
                        r1_pool.tile([P, QK, BL], FP8, tag="r2qb", name="r2qb"))
            r2 = []
            for j in range(JC):
                ps2 = ps_pair("ps_r2_")
                qq, jj = divmod(j, QK)
                t_ = None
                if j >= n8:
                    t_ = r2_pool.tile([P, BL], BF16, tag=f"r2_{j}",
                                      name=f"r2_{j}")
                    r2.append(t_)
                for bh in range(2):
                    mv = slice(bh * NH, (bh + 1) * NH)
                    for t in range(KC1 // 2):
                        nc.tensor.matmul(
                            ps2[bh][:], r2big[:, j, 2 * t:2 * t + 2, :],
                            r1q[t // 2][:, (2 * t) % QK:(2 * t) % QK + 2, mv],
                            start=(t == 0), stop=(t == KC1 // 2 - 1),
                            perf_mode=DR)
                    dst = (r2q8[qq][:, jj, mv] if j < n8 else t_[:, mv])
                    nc.scalar.activation(dst, ps2[bh][:], AF.Relu,
                                         bias=bias_ap(7, j), scale=1.0 / WS)

            # ---- phase B: gates + r3 + combine, per feature tile j.
            # Order c,s,i (fp8), o, f (bf16), r3: the elementwise chain runs
            # while later matmuls stream; o comes before f/r3 so only the
            # short r3-evict -> add -> tanh -> mul chain trails the last MM.
            GATE8 = {"c": (0, 3, AF.Tanh), "s": (1, 4, AF.Sigmoid),
                     "i": (2, 0, AF.Sigmoid)}

            def gate8(key, j):
                gi, v, fn = GATE8[key]
                wt = load_w8(gi, j)
                t_ = g_pool.tile([P, BL], F32, tag=f"g8{key}", name=f"g8{key}")
                ps2 = ps_pair("ps_g8")

                def ev(bh):
                    mv = slice(bh * NH, (bh + 1) * NH)
                    nc.scalar.activation(t_[:, mv], ps2[bh][:], fn,
                                         bias=bias_ap(v, j), scale=1.0 / WS)
                mm8(ps2, wt, evict=ev)
                return t_

            def gate16(gi, v, j):
                wt = load_wg16(gi, j)
                t_ = g_pool.tile([P, BL], F32, tag=f"g16{gi}", name=f"g16{gi}")
                ps2 = ps_pair("ps_g16")

                def ev(bh):
                    mv = slice(bh * NH, (bh + 1) * NH)
                    nc.scalar.activation(t_[:, mv], ps2[bh][:], AF.Sigmoid,
                                         bias=bias_ap(v, j))
                mm16(ps2, wt, rhs_xh16, KC2, evict=ev)
                return t_

            def rhs_r2q8(t, mv):
                # DR pair t over the fp8 r2 quarters
                return r2q8[t // 2][:, (2 * t) % QK:(2 * t) % QK + 2, mv]

            def rhs_r2_16(k, mv):
                # bf16 r2 tiles; in half8 mode tile list starts at k=QK
                return r2[k - (QK if r3_mode == "half8" else 0)][:, mv]

            def mm_r3(ps2, j, wt3b, evict):
                # r3 accumulation: fp8 DR pairs then bf16 k-tiles (both
                # weight halves pre-scaled by WS so PSUM shares one scale)
                n_dr = {"full8": KC1 // 2, "half8": KC1 // 4, "bf16": 0}[r3_mode]
                n_16 = KC1 - 2 * n_dr
                for bh in range(2):
                    mv = slice(bh * NH, (bh + 1) * NH)
                    for t in range(n_dr):
                        nc.tensor.matmul(
                            ps2[bh][:], r3big[:, j, 2 * t:2 * t + 2, :],
                            rhs_r2q8(t, mv), start=(t == 0),
                            stop=(t == n_dr - 1 and n_16 == 0),
                            perf_mode=DR)
                    for k in range(n_16):
                        nc.tensor.matmul(
                            ps2[bh][:], wt3b[:, k * P:(k + 1) * P],
                            rhs_r2_16(2 * n_dr + k, mv),
                            start=(k == 0 and n_dr == 0),
                            stop=(k == n_16 - 1))
                    evict(bh)

            for j in range(JC):
                last = j == JC - 1
                ch = gate8("c", j)
                st = gate8("s", j)
                it = gate8("i", j)

                t1s = []
                for bh in range(2):
                    mv = slice(bh * NH, (bh + 1) * NH)
                    t1 = ew_pool.tile([P, NH], F32, tag=f"t1{bh}", name="t1")
                    nc.vector.tensor_mul(t1[:], it[:, mv], ch[:, mv])
                    nc.vector.tensor_mul(t1[:], t1[:], st[:, mv])
                    nc.vector.tensor_mul(t1[:], t1[:], alpha_rep[:, mv])
                    t1s.append(t1)

                # last j: f before o, so f's evict -> t2 -> t1 chain
                # hides under o's and r3's matmuls and only the short
                # r3-evict -> add -> tanh -> mul chain trails the last MM
                if last:
                    ft = gate16(0, 1, j)
                    ot = gate16(1, 2, j)
                else:
                    ot = gate16(1, 2, j)
                    ft = gate16(0, 1, j)
                for bh in range(2):
                    mv = slice(bh * NH, (bh + 1) * NH)
                    qq, kk = divmod(j, QK)
                    t2 = ew_pool.tile([P, NH], F32, tag=f"t2{bh}", name="t2",
                                      bufs=1)
                    nc.vector.tensor_mul(t2[:], ft[:, mv],
                                         cpq[qq][:, kk, mv])
                    nc.vector.tensor_add(t1s[bh][:], t1s[bh][:], t2[:])

                if r3_mode == "bf16":
                    wt3b = wf_pool.tile([P, D], BF16, tag="wr", bufs=3,
                                        name=f"wr3_{j}")
                    nc.sync.dma_start(out=wt3b[:], in_=wr3[j])
                elif r3_mode == "half8":
                    wt3b = wf_pool.tile([P, (KC1 // 2) * P], BF16, tag="wr",
                                        bufs=3, name=f"wr3h_{j}")
                    nc.sync.dma_start(out=wt3b[:], in_=wr3h[j])
                else:
                    wt3b = None
                ps2 = ps_pair("ps_r3_")
                # stage[:, 0, :] = h, stage[:, 1, :] = c -> single store.
                # The finish chain runs per batch half right after that
                # half's accumulation stops (overlapping the other
                # half's matmuls); the last j additionally splits into
                # NH/2 chunks to pipeline evict -> add -> tanh -> mul.
                r3scale = 1.0 if r3_mode == "bf16" else 1.0 / WS
                nch = 1 if not last else 2
                hw_ = NH // nch

                def finish(bh):
                    for cc in range(nch):
                        pv = slice(cc * hw_, (cc + 1) * hw_)
                        mv = slice(bh * NH + cc * hw_,
                                   bh * NH + (cc + 1) * hw_)
                        stg = ew_pool.tile([P, 2, hw_], BF16,
                                           tag=f"st{bh}_{nch}{cc}", name="stg")
                        if r3_mode == "bf16":
                            # c = (r3_psum + r3_bias) + t1 in one DVE op
                            # straight from PSUM
                            nc.vector.scalar_tensor_tensor(
                                stg[:, 1, :], ps2[bh][:, pv], bias_ap(8, j),
                                t1s[bh][:, pv],
                                mybir.AluOpType.add, mybir.AluOpType.add)
                        else:
                            # fp8 r3 PSUM carries the WS weight scale:
                            # evict via ScalarE (scale+bias), add on DVE
                            r3sb = ew_pool.tile([P, hw_],
                                                F32 if r3_mode == "full8"
                                                else BF16,
                                                tag=f"r3{bh}{cc}", name="r3sb",
                                                bufs=1)
                            nc.scalar.activation(r3sb[:], ps2[bh][:, pv],
                                                 AF.Identity,
                                                 bias=bias_ap(8, j),
                                                 scale=r3scale)
                            nc.vector.tensor_add(stg[:, 1, :], t1s[bh][:, pv],
                                                 r3sb[:])
                        th = ew_pool.tile([P, hw_],
                                          F32 if r3_mode == "full8" else BF16,
                                          tag=f"th{bh}{cc}",
                                          name="th", bufs=1)
                        nc.scalar.activation(th[:], stg[:, 1, :], AF.Tanh)
                        nc.vector.tensor_mul(stg[:, 0, :], ot[:, mv], th[:])
                        # last j's stores ride Sync (idle by then): a
                        # trailing gpsimd store chain made the epilogue
                        # queue-drain ~5us long
                        eng = nc.sync if last else nc.gpsimd
                        eng.dma_start(
                            out=out[j * P:(j + 1) * P, :, mv], in_=stg[:])
                mm_r3(ps2, j, wt3b, evict=finish)

    nc.finalize()
    return nc


def _pack_w(W, kdim):
    # pack[j, p, k*128+m] = W[j*128+m, k*128+p]
    kc = kdim // P
    return np.ascontiguousarray(
        np.asarray(W, np.float32).reshape(JC, P, kc, P)
        .transpose(0, 3, 2, 1).reshape(JC, P, kc * P))


def _pack_act(aT, nq, qk=QK):
    # aT: [nq*qk*P, BL] -> [nq, P, qk, BL] with [q, p, kk, n] = aT[(q*qk+kk)*P+p, n]
    return np.ascontiguousarray(
        aT.reshape(nq, qk, P, BL).transpose(0, 2, 1, 3))


def _prepare(inputs, r3_mode):
    f = lambda name: np.asarray(inputs[name], dtype=np.float32)

    def comb(g):
        u = "U" + g[1]
        return np.concatenate([f(g + "_w"), f(u + "_w")], axis=1)

    # fp8 gates: c, s, i, a1 (order matches in-kernel GATE8/a1 indices)
    w8 = np.stack([
        _pack_w(comb("Wc") * WS, K2),
        _pack_w(comb("Ws") * WS, K2),
        _pack_w(comb("Wi") * WS, K2),
        _pack_w(f("a1_w") * WS, K2),
    ]).astype(E4NP).reshape(4, JC, P, KC2, P)
    def pack_pfirst(W):
        # [P, JC, KC1, P]: partition dim first so the whole matrix is
        # one contiguous-per-partition DMA
        return np.ascontiguousarray(
            _pack_w(W, D).reshape(JC, P, KC1, P).transpose(1, 0, 2, 3))

    w8r1 = pack_pfirst(f("r1_w") * WS).astype(E4NP)
    w8r2 = pack_pfirst(f("r2_w") * WS).astype(E4NP)
    # bf16 gates: f, o
    wf_ = np.stack([_pack_w(comb("Wf"), K2),
                    _pack_w(comb("Wo"), K2)]).astype(BFNP)
    a2p = np.zeros((P, KC1, 16), np.float32)
    a2p[:, :, 0] = (f("a2_w") * WS).reshape(KC1, P).T
    a2p = a2p.astype(E4NP)

    shared = {"w8": w8, "w8r1": w8r1, "w8r2": w8r2, "wf": wf_, "a2p": a2p}
    r3pf = pack_pfirst(f("r3_w") * WS)  # [P, JC, KC1, P], scaled by WS
    if r3_mode == "full8":
        shared["w8r3"] = r3pf.astype(E4NP)
    elif r3_mode == "half8":
        shared["w8r3"] = np.ascontiguousarray(
            r3pf[:, :, :KC1 // 2]).astype(E4NP)
        shared["wr3h"] = np.ascontiguousarray(
            r3pf.transpose(1, 0, 2, 3)[:, :, KC1 // 2:]
            .reshape(JC, P, (KC1 // 2) * P)).astype(BFNP)
    else:
        shared["wr3"] = _pack_w(f("r3_w"), D).astype(BFNP)

    bias_vecs = []
    for g in ("Wi", "Wf", "Wo", "Wc", "Ws"):
        u = "U" + g[1]
        bias_vecs.append(f(g + "_b") + f(u + "_b"))
    bias_vecs += [f("a1_b"), f("r1_b"), f("r2_b"), f("r3_b"),
                  np.full(D, f("a2_b")[0], np.float32)]
    # biasp[p, v*JC + j] = vec_v[j*128 + p]
    biasp = np.ascontiguousarray(
        np.stack(bias_vecs).reshape(10, JC, P).transpose(2, 0, 1)
        .reshape(P, 10 * JC))
    shared["biasp"] = biasp

    x, h, c = f("x"), f("h_prev"), f("c_prev")
    in_maps = []
    for core in range(NCORES):
        sl = slice(core * BL, (core + 1) * BL)
        xhT = np.ascontiguousarray(
            np.concatenate([x[sl].T, h[sl].T], axis=0))  # [K2, BL]
        in_maps.append({**shared,
                        "xh16s": _pack_act(xhT.astype(BFNP), 4),
                        "xh8s": _pack_act(xhT.astype(E4NP), 4),
                        "cTs": _pack_act(
                            np.ascontiguousarray(c[sl].T).astype(BFNP), 2)})
    return in_maps


def _run(inputs, trace=False):
    from concourse.bass_utils import run_bass_kernel_spmd

    if R3_MODE not in _CACHE:
        _CACHE[R3_MODE] = _build(R3_MODE)
    nc = _CACHE[R3_MODE]
    in_maps = _prepare(inputs, R3_MODE)
    res = run_bass_kernel_spmd(nc, in_maps, core_ids=list(range(NCORES)),
                               trace=trace)
    h = np.empty((B, D), np.float32)
    c = np.empty((B, D), np.float32)
    for core in range(NCORES):
        o = res.results[core]["out"]  # [D, 2, BL] bf16
        sl = slice(core * BL, (core + 1) * BL)
        h[sl] = o[:, 0].T.astype(np.float32)
        c[sl] = o[:, 1].T.astype(np.float32)
    return (h, c), res


def kernel(**inputs):
    (h, c), _ = _run(inputs, trace=False)
    return (h, c)
